# revision 31
# baseline (speedup 1.0000x reference)
"""Trainium2 Bass kernel for CrossAttention (B=2, N=M=2048, 16 heads x 64).

Sharding: batch x head-group parallel over 8 cores. Core c handles batch
c//4 and heads [4*(c%4), 4*(c%4)+4). Projection weights are column-split
(Wq/Wk/Wv) / row-split (Wo) per core; each core produces a partial
[2048, 1024] output (bf16) which the host sums per batch (4 partials).

V2 design (cost-model driven):
  - All DRAM inputs and SBUF matmul operands are bf16 (same 1 cycle/row
    PE speed as f32r, half the DMA bytes, ~5e-3 rel err total).
  - ACT engine runs ONLY the softmax Exp (131072 rows = the 133us floor);
    all DMAs/copies live on SP/DVE/Pool queues.
  - PE is the bottleneck (393216 matmul rows = 163.8us floor).  Emission
    keeps PE saturated: KT -> QT(qc0) -> per q-chunk/head-pair S/exp/O
    streams with "fill" matmuls (QT for later q-chunks, fused output
    projection of earlier q-chunks, V during the first chunk) interleaved
    one per m-tile to absorb the exp-vs-PE rate gap.
  - Normalization: ones column in V_aug yields denominators in PSUM row
    64; DVE reciprocal, SP DMA to partition 0, gpsimd partition_broadcast,
    DVE multiply (bf16 out).  PSUM: 2x s_t (2 banks each) + 3x o_t +
    1 fill bank = 8 banks.
"""

import numpy as np
import ml_dtypes
from contextlib import ExitStack

import concourse.tile as tile
from concourse import bacc, mybir
from concourse.bass_utils import run_bass_kernel_spmd

B, N, M, C = 2, 2048, 2048, 1024
HEADS, D = 16, 64
HPC = 4            # heads per core
IC = HPC * D       # 256 inner dims per core
SCALE = D ** -0.5
NCORES = 8
KT_TILES = C // 128   # 8 contraction tiles for projections
P = 128
MT = M // P           # 16 m tiles
QC = 512
NQC = N // QC         # 4 q chunks
f32 = mybir.dt.float32
bf16 = mybir.dt.bfloat16

_CACHE = {}


def _body(nc, tc, ctx, xd, ctxd, wq, wk, wv, wo, out, opt=None):
    opt = opt or {}
    ES_BUFS = opt.get("es_bufs", 4)
    O_BUFS = opt.get("o_bufs", 2)
    F_BUFS = opt.get("f_bufs", 2)
    BCAST64 = opt.get("bcast64", False)
    GP_AOPACK = opt.get("gp_aopack", True)
    ONES0 = opt.get("ones0", True)

    const = ctx.enter_context(tc.tile_pool(name="const", bufs=1))
    wq_sb = const.tile([P, KT_TILES, IC], bf16, tag="wq")
    wk_sb = const.tile([P, KT_TILES, IC], bf16, tag="wk")
    wv_sb = const.tile([P, KT_TILES, IC], bf16, tag="wv")
    wo_sb = const.tile([P, 2, C], bf16, tag="wo")
    ctx_sb = const.tile([P, KT_TILES, M], bf16, tag="ctx")
    x_sb = const.tile([P, KT_TILES, N], bf16, tag="x")
    kt_sb = [const.tile([P, M], bf16, tag=f"kt{j}", name=f"kt{j}") for j in range(2)]
    qt_sb = [const.tile([P, N], bf16, tag=f"qt{j}", name=f"qt{j}") for j in range(2)]
    VW = 2 * D  # ones | zeros pad | V values
    v_sb = const.tile([P, MT, HPC, VW], bf16, tag="v")
    ao_sb = [const.tile([P, N], bf16, tag=f"ao{j}", name=f"ao{j}") for j in range(2)]
    ones_sb = const.tile([P, 1], f32, tag="ones")

    es_pool = ctx.enter_context(tc.tile_pool(name="es", bufs=ES_BUFS))
    r_pool = ctx.enter_context(tc.tile_pool(name="rp", bufs=2))
    rb_pool = ctx.enter_context(tc.tile_pool(name="rbp", bufs=2))
    oc_pool = ctx.enter_context(tc.tile_pool(name="ocp", bufs=4))
    aot_pool = ctx.enter_context(tc.tile_pool(name="aot", bufs=2))
    outst_pool = ctx.enter_context(tc.tile_pool(name="outst", bufs=4))

    junk_sb = const.tile([P, QC], bf16, tag="junk")

    # junk tile first on DVE so PE warmup matmuls can start ~1us in
    nc.vector.memset(junk_sb[:], 0.0)
    # ones column of V_aug (f32 memset + broadcast-copy cast to bf16)
    nc.vector.memset(ones_sb[:], 1.0)
    nc.gpsimd.memset(v_sb[:, :, :, 1:D], 0.0)
    nc.vector.tensor_copy(
        v_sb[:, :, :, 0:1],
        ones_sb[:, 0:1].to_broadcast((P, MT, HPC, 1)),
    )

    def emit_junk(pool, n, tag="warm"):
        # PE keep-warm matmuls (nothing reads the result): bridge idle
        # windows so the pstate ramp does not reset.
        jp = pool.tile([P, QC], f32, tag=tag, name=f"junk_{tag}")
        for _ in range(n):
            nc.tensor.matmul(
                jp[:], junk_sb[0:P, 0:P], junk_sb[:, 0:QC],
                start=True, stop=True,
            )

    # ---- input DMAs ----
    # ACT: weights + x qc0 (all done before the first exp); SP: ctx + x rest
    # first wk k-tile + ctx k0 quarters split so the first KT matmul starts early
    for k in range(KT_TILES):
        nc.scalar.dma_start(wq_sb[:, k, :], wq[:, k, :])
        nc.scalar.dma_start(x_sb[:, k, 0:QC], xd[:, k, 0:QC])
        nc.scalar.dma_start(wk_sb[:, k, :], wk[:, k, :])
    for k in range(KT_TILES):
        eng = nc.sync if k % 2 == 0 else nc.gpsimd
        eng.dma_start(ctx_sb[:, k, :], ctxd[:, k, :])
    nc.scalar.dma_start(wv_sb[:], wv[:])
    nc.scalar.dma_start(wo_sb[:], wo[:])
    for k in range(KT_TILES):
        nc.sync.dma_start(x_sb[:, k, QC:N], xd[:, k, QC:N])

    def emit_V(m, pool):
        vt = pool.tile([P, QC], f32, tag=pool_tag[id(pool)], name=f"vt{m}")
        for k in range(KT_TILES):
            nc.tensor.matmul(
                vt[:, 0:IC],
                ctx_sb[:, k, m * P:(m + 1) * P],
                wv_sb[:, k, :],
                start=(k == 0), stop=(k == KT_TILES - 1),
            )
        nc.vector.tensor_copy(
            v_sb[:, m, :, D:VW],
            vt[:, 0:IC].rearrange("p (h d) -> p h d", d=D),
        )

    pool_tag = {}

    # ---- PE warmup, then projections: KT (8 psum banks, k-outer), QT qc0 ----
    with tc.tile_pool(name="warm_ps", bufs=1, space="PSUM") as wps:
        emit_junk(wps, 6, tag="warm0")

    with tc.tile_pool(name="pp8", bufs=8, space="PSUM") as pps:
        pool_tag[id(pps)] = "pp"
        # QT qc0 (both j) and KT j0 interleaved per k-tile, tracking the
        # per-k weight/x/ctx DMA arrival order.
        qp = [pps.tile([P, QC], f32, tag="pp", name=f"qp{j}") for j in range(2)]
        kp0 = [pps.tile([P, QC], f32, tag="pp", name=f"kp0_{qc}") for qc in range(4)]
        for k in range(KT_TILES):
            for j in range(2):
                nc.tensor.matmul(
                    qp[j][:],
                    wq_sb[:, k, j * P:(j + 1) * P],
                    x_sb[:, k, 0:QC],
                    start=(k == 0), stop=(k == KT_TILES - 1),
                )
            for qc in range(4):
                nc.tensor.matmul(
                    kp0[qc][:],
                    wk_sb[:, k, 0:P],
                    ctx_sb[:, k, qc * QC:(qc + 1) * QC],
                    start=(k == 0), stop=(k == KT_TILES - 1),
                )
        nc.vector.tensor_copy(qt_sb[0][:, 0:QC], qp[0][:])
        nc.vector.tensor_copy(qt_sb[1][:, 0:QC], qp[1][:])
        for qc in range(4):
            nc.vector.tensor_copy(kt_sb[0][:, qc * QC:(qc + 1) * QC], kp0[qc][:])
        # KT j1 qc-major (ctx fully resident by now); last two chunks reuse
        # the QT banks, drained by the qt copies above.
        for qc in range(4):
            kp1 = pps.tile([P, QC], f32, tag="pp", name=f"kp1_{qc}")
            for k in range(KT_TILES):
                nc.tensor.matmul(
                    kp1[:],
                    wk_sb[:, k, P:2 * P],
                    ctx_sb[:, k, qc * QC:(qc + 1) * QC],
                    start=(k == 0), stop=(k == KT_TILES - 1),
                )
            nc.vector.tensor_copy(kt_sb[1][:, qc * QC:(qc + 1) * QC], kp1[:])
        # V[0]: covers the kt j1 copies + attention pool transition
        emit_V(0, pps)

    # ---- attention with interleaved fill work ----
    with ExitStack() as attn_ctx:
        sps = attn_ctx.enter_context(tc.tile_pool(name="s_ps", bufs=2, space="PSUM"))
        ops = attn_ctx.enter_context(tc.tile_pool(name="o_ps", bufs=O_BUFS, space="PSUM"))
        fps = attn_ctx.enter_context(tc.tile_pool(name="f_ps", bufs=F_BUFS, space="PSUM"))
        pool_tag[id(fps)] = "f"

        # QT fill state: one matmul per fill slot, j-serial per q-chunk
        qt_state = {}

        def emit_QT(qcn, slot):
            j, k = slot // KT_TILES, slot % KT_TILES
            if k == 0:
                qt_state[(qcn, j)] = fps.tile([P, QC], f32, tag="f", name=f"qf{qcn}_{j}")
            qtile = qt_state[(qcn, j)]
            nc.tensor.matmul(
                qtile[:],
                wq_sb[:, k, j * P:(j + 1) * P],
                x_sb[:, k, qcn * QC:(qcn + 1) * QC],
                start=(k == 0), stop=(k == KT_TILES - 1),
            )
            if k == KT_TILES - 1:
                nc.vector.tensor_copy(qt_sb[j][:, qcn * QC:(qcn + 1) * QC], qtile[:])
                del qt_state[(qcn, j)]

        # final projection fill: slot -> (nt, ec, j)
        fin_state = {}

        def emit_FIN(qcn, slot, pool):
            nt = qcn * 4 + slot // 4
            ec = (slot // 2) % 2
            j = slot % 2
            if j == 0:
                fin_state["ft"] = pool.tile(
                    [P, QC], f32, tag=pool_tag[id(pool)], name=f"ft{nt}_{ec}")
            ft = fin_state["ft"]
            nc.tensor.matmul(
                ft[:],
                ao_sb[j][:, nt * P:(nt + 1) * P],
                wo_sb[:, j, ec * QC:(ec + 1) * QC],
                start=(j == 0), stop=(j == 1),
            )
            if j == 0 and ec == 0:
                fin_state["ostg"] = outst_pool.tile([P, C], bf16, tag="ostg",
                                                    name=f"og{nt}")
            if j == 1:
                nc.vector.tensor_copy(fin_state["ostg"][:, ec * QC:(ec + 1) * QC], ft[:])
                if ec == 1:
                    deng = nc.sync if nt % 2 == 0 else nc.gpsimd
                    deng.dma_start(out[nt * P:(nt + 1) * P, :], fin_state["ostg"][:])

        # fill plan per (qc, p) segment
        def fill(qc, p, mt):
            if qc == 0 and p == 0:
                if mt + 1 < MT:
                    emit_V(mt + 1, fps)
            elif qc == 0 and p == 1:
                emit_QT(1, mt)
            elif qc == 1 and p == 0:
                emit_QT(2, mt)
            elif qc == 1 and p == 1:
                emit_FIN(0, mt, fps)
            elif qc == 2 and p == 0:
                emit_QT(3, mt)
            elif qc == 2 and p == 1:
                emit_FIN(1, mt, fps)
            elif qc == 3 and p == 0:
                emit_FIN(2, mt, fps)
            # (3,1): no fill available

        def normalize(qc, p, o_ts):
            # Chain (hh1 first; it gates the fused output projection):
            #   DVE: recip1 (straight from PSUM), oc1 copy, mul1, recip0, mul0
            #   Pool: bcast1, oc0 copy, bcast0
            # o banks freed by {recip, oc copy}; ao written by muls/pack-DMA.
            q0 = qc * QC
            rs, rbs, ocs = {}, {}, {}
            for hh in (1, 0):
                rs[hh] = r_pool.tile([P, QC], f32, tag="r", name=f"r{qc}_{p}_{hh}")
                rbs[hh] = rb_pool.tile([P, QC], f32, tag="rb", name=f"rb{qc}_{p}_{hh}")
                ocs[hh] = oc_pool.tile([P, QC], f32, tag="oc", name=f"oc{qc}_{p}_{hh}")

            def bcast(hh):
                if BCAST64:
                    nc.gpsimd.partition_broadcast(rbs[hh][0:D, :], rs[hh][64:65, :])
                else:
                    nc.sync.dma_start(rs[hh][0:1, :], rs[hh][64:65, :])
                    nc.gpsimd.partition_broadcast(rbs[hh][0:D, :], rs[hh][0:1, :])

            nc.vector.tensor_copy(ocs[1][0:1, :], o_ts[1][0:1, :])
            nc.vector.tensor_copy(ocs[1][D:2 * D, :], o_ts[1][D:2 * D, :])
            nc.vector.tensor_copy(ocs[0][0:1, :], o_ts[0][0:1, :])
            nc.vector.tensor_copy(ocs[0][D:2 * D, :], o_ts[0][D:2 * D, :])
            nc.vector.reciprocal(rs[1][0:1, :], ocs[1][0:1, :])
            nc.vector.reciprocal(rs[0][0:1, :], ocs[0][0:1, :])
            nc.gpsimd.partition_broadcast(rbs[1][0:D, :], rs[1][0:1, :])
            nc.gpsimd.partition_broadcast(rbs[0][0:D, :], rs[0][0:1, :])
            nc.gpsimd.tensor_mul(
                ao_sb[p][64:P, q0:q0 + QC], ocs[1][D:2 * D, :], rbs[1][0:D, :]
            )
            nc.gpsimd.tensor_mul(
                ao_sb[p][0:D, q0:q0 + QC], ocs[0][D:2 * D, :], rbs[0][0:D, :]
            )

        for qc in range(NQC):
            q0 = qc * QC
            for p in range(2):
                o_ts = [ops.tile([P, QC], f32, tag="o", name=f"o{qc}_{p}_{i}")
                        for i in range(2)]
                if qc == 0 and p == 0:
                    pass  # V[0], V[1] were emitted in the projection scope
                for mt in range(MT):
                    s_t = sps.tile([P, 2 * QC], f32, tag="s", name=f"s{qc}_{p}_{mt}")
                    for hh in range(2):
                        pb = hh * 64
                        nc.tensor.matmul(
                            s_t[:, hh * QC:(hh + 1) * QC],
                            kt_sb[p][pb:pb + 64, mt * P:(mt + 1) * P],
                            qt_sb[p][pb:pb + 64, q0:q0 + QC],
                            start=True, stop=True,
                        )
                    es = es_pool.tile([P, 2 * QC], bf16, tag="es", name=f"es{qc}_{p}_{mt}")
                    nc.scalar.activation(
                        es[:], s_t[:],
                        mybir.ActivationFunctionType.Exp, scale=SCALE,
                    )
                    fill(qc, p, mt)
                    for hh in range(2):
                        h = 2 * p + hh
                        nc.tensor.matmul(
                            o_ts[hh][:],
                            v_sb[:, mt, h, :],
                            es[:, hh * QC:(hh + 1) * QC],
                            start=(mt == 0), stop=(mt == MT - 1),
                        )
                normalize(qc, p, o_ts)

    # ---- tail: final projection for qc3 ----
    # ao p0 is ready well before ao p1 (its normalize ends the kernel), so:
    # j0 accumulation steps first (4 open banks), junk bridge keeps the PE
    # pstate warm while normalize(qc3,p1) completes, then the j1 steps.
    with (
        tc.tile_pool(name="tail_ps", bufs=7, space="PSUM") as tps,
        tc.tile_pool(name="tailj_ps", bufs=1, space="PSUM") as tjp,
    ):
        fts = {}
        ostgs = {}

        def fin3_mm(i, j):
            nt = 12 + i // 2
            ec = i % 2
            if j == 0:
                fts[i] = tps.tile([P, QC], f32, tag="tf", name=f"tf{i}")
            nc.tensor.matmul(
                fts[i][:],
                ao_sb[j][:, nt * P:(nt + 1) * P],
                wo_sb[:, j, ec * QC:(ec + 1) * QC],
                start=(j == 0), stop=(j == 1),
            )
            if j == 0 and ec == 0:
                ostgs[nt] = outst_pool.tile([P, C], bf16, tag="ostg", name=f"og{nt}")
            if j == 1:
                nc.vector.tensor_copy(ostgs[nt][:, ec * QC:(ec + 1) * QC], fts[i][:])
                deng = nc.sync if ec == 1 else nc.gpsimd
                deng.dma_start(out[nt * P:(nt + 1) * P, ec * QC:(ec + 1) * QC],
                               ostgs[nt][:, ec * QC:(ec + 1) * QC])

        for i in range(4):
            fin3_mm(i, 0)
        emit_junk(tjp, opt.get("tail_junk", 13), tag="warm1")
        for i in range(4):
            fin3_mm(i, 1)
        for i in range(4, 8):
            fin3_mm(i, 0)
        for i in (6, 7, 4, 5):   # last row group first so its copy+DMA drain early
            fin3_mm(i, 1)


def _build(reps=1, opt=None):
    key = (reps, tuple(sorted((opt or {}).items())))
    if key in _CACHE:
        return _CACHE[key]
    nc = bacc.Bacc("TRN2", target_bir_lowering=False, debug=False)
    xd = nc.dram_tensor("xd", [P, KT_TILES, N], bf16, kind="ExternalInput")
    ctxd = nc.dram_tensor("ctxd", [P, KT_TILES, M], bf16, kind="ExternalInput")
    wq = nc.dram_tensor("wq", [P, KT_TILES, IC], bf16, kind="ExternalInput")
    wk = nc.dram_tensor("wk", [P, KT_TILES, IC], bf16, kind="ExternalInput")
    wv = nc.dram_tensor("wv", [P, KT_TILES, IC], bf16, kind="ExternalInput")
    wo = nc.dram_tensor("wo", [P, 2, C], bf16, kind="ExternalInput")
    out = nc.dram_tensor("out", [N, C], bf16, kind="ExternalOutput")
    with tile.TileContext(nc) as tc:
        for _ in range(reps):
            with ExitStack() as ctx:
                _body(nc, tc, ctx, xd, ctxd, wq, wk, wv, wo, out, opt=opt)
    nc.compile()
    _CACHE[key] = nc
    return nc


def _to_tiled(a, inner):
    """[K*128, inner] f32 -> [128, K, inner] bf16 (partition-major tiling)."""
    k = a.shape[0] // P
    return np.ascontiguousarray(
        a.reshape(k, P, inner).transpose(1, 0, 2).astype(ml_dtypes.bfloat16)
    )


def _shard_inputs(x, context, Wq, Wk, Wv, Wo):
    in_maps = []
    for c in range(NCORES):
        b, g = divmod(c, NCORES // B)
        cols = slice(g * IC, (g + 1) * IC)
        in_maps.append({
            "xd": _to_tiled(np.ascontiguousarray(x[b].T), N),
            "ctxd": _to_tiled(np.ascontiguousarray(context[b].T), M),
            "wq": _to_tiled(np.ascontiguousarray(Wq[:, cols]), IC),
            "wk": _to_tiled(np.ascontiguousarray(Wk[:, cols]), IC),
            "wv": _to_tiled(np.ascontiguousarray(Wv[:, cols]), IC),
            "wo": _to_tiled(np.ascontiguousarray(Wo[cols, :]), C),
        })
    return in_maps


def kernel(x, context, Wq, Wk, Wv, Wo, reps=1):
    x = np.asarray(x, dtype=np.float32)
    context = np.asarray(context, dtype=np.float32)
    Wq, Wk, Wv, Wo = (np.asarray(w, dtype=np.float32) for w in (Wq, Wk, Wv, Wo))
    nc = _build(reps)
    in_maps = _shard_inputs(x, context, Wq, Wk, Wv, Wo)
    res = run_bass_kernel_spmd(nc, in_maps, core_ids=list(range(NCORES)))
    gpb = NCORES // B
    out = np.zeros((B, N, C), dtype=np.float32)
    for c in range(NCORES):
        out[c // gpb] += np.asarray(res.results[c]["out"], dtype=np.float32)
    return out


# revision 32
# speedup vs baseline: 1.0294x; 1.0294x over previous
"""Trainium2 Bass kernel for CrossAttention (B=2, N=M=2048, 16 heads x 64).

Sharding: batch x head-group parallel over 8 cores. Core c handles batch
c//4 and heads [4*(c%4), 4*(c%4)+4). Projection weights are column-split
(Wq/Wk/Wv) / row-split (Wo) per core; each core produces a partial
[2048, 1024] output (bf16) which the host sums per batch (4 partials).

V2 design (cost-model driven):
  - All DRAM inputs and SBUF matmul operands are bf16 (same 1 cycle/row
    PE speed as f32r, half the DMA bytes, ~5e-3 rel err total).
  - ACT engine runs ONLY the softmax Exp (131072 rows = the 133us floor);
    all DMAs/copies live on SP/DVE/Pool queues.
  - PE is the bottleneck (393216 matmul rows = 163.8us floor).  Emission
    keeps PE saturated: KT -> QT(qc0) -> per q-chunk/head-pair S/exp/O
    streams with "fill" matmuls (QT for later q-chunks, fused output
    projection of earlier q-chunks, V during the first chunk) interleaved
    one per m-tile to absorb the exp-vs-PE rate gap.
  - Normalization: ones column in V_aug yields denominators in PSUM row
    64; DVE reciprocal, SP DMA to partition 0, gpsimd partition_broadcast,
    DVE multiply (bf16 out).  PSUM: 2x s_t (2 banks each) + 3x o_t +
    1 fill bank = 8 banks.
"""

import numpy as np
import ml_dtypes
from contextlib import ExitStack

import concourse.tile as tile
from concourse import bacc, mybir
from concourse.bass_utils import run_bass_kernel_spmd

B, N, M, C = 2, 2048, 2048, 1024
HEADS, D = 16, 64
HPC = 4            # heads per core
IC = HPC * D       # 256 inner dims per core
SCALE = D ** -0.5
NCORES = 8
KT_TILES = C // 128   # 8 contraction tiles for projections
P = 128
MT = M // P           # 16 m tiles
QC = 512
NQC = N // QC         # 4 q chunks
f32 = mybir.dt.float32
bf16 = mybir.dt.bfloat16

_CACHE = {}


def _body(nc, tc, ctx, xd, ctxd, wq, wk, wv, wo, out, opt=None):
    opt = opt or {}
    ES_BUFS = opt.get("es_bufs", 4)
    O_BUFS = opt.get("o_bufs", 2)
    F_BUFS = opt.get("f_bufs", 2)
    BCAST64 = opt.get("bcast64", False)
    GP_AOPACK = opt.get("gp_aopack", True)
    ONES0 = opt.get("ones0", True)

    const = ctx.enter_context(tc.tile_pool(name="const", bufs=1))
    wq_sb = const.tile([P, KT_TILES, IC], bf16, tag="wq")
    wk_sb = const.tile([P, KT_TILES, IC], bf16, tag="wk")
    wv_sb = const.tile([P, KT_TILES, IC], bf16, tag="wv")
    wo_sb = const.tile([P, 2, C], bf16, tag="wo")
    ctx_sb = const.tile([P, KT_TILES, M], bf16, tag="ctx")
    x_sb = const.tile([P, KT_TILES, N], bf16, tag="x")
    kt_sb = [const.tile([P, M], bf16, tag=f"kt{j}", name=f"kt{j}") for j in range(2)]
    qt_sb = [const.tile([P, N], bf16, tag=f"qt{j}", name=f"qt{j}") for j in range(2)]
    VW = 2 * D  # ones | zeros pad | V values
    v_sb = const.tile([P, MT, HPC, VW], bf16, tag="v")
    ao_sb = [const.tile([P, N], bf16, tag=f"ao{j}", name=f"ao{j}") for j in range(2)]
    ones_sb = const.tile([P, 1], f32, tag="ones")

    es_pool = ctx.enter_context(tc.tile_pool(name="es", bufs=ES_BUFS))
    r_pool = ctx.enter_context(tc.tile_pool(name="rp", bufs=2))
    rb_pool = ctx.enter_context(tc.tile_pool(name="rbp", bufs=2))
    oc_pool = ctx.enter_context(tc.tile_pool(name="ocp", bufs=4))
    aot_pool = ctx.enter_context(tc.tile_pool(name="aot", bufs=2))
    outst_pool = ctx.enter_context(tc.tile_pool(name="outst", bufs=4))

    junk_sb = const.tile([P, QC], bf16, tag="junk")

    # junk tile first on DVE so PE warmup matmuls can start ~1us in
    nc.vector.memset(junk_sb[:], 0.0)
    # ones column of V_aug (f32 memset + broadcast-copy cast to bf16)
    nc.vector.memset(ones_sb[:], 1.0)
    nc.gpsimd.memset(v_sb[:, :, :, 1:D], 0.0)
    nc.vector.tensor_copy(
        v_sb[:, :, :, 0:1],
        ones_sb[:, 0:1].to_broadcast((P, MT, HPC, 1)),
    )

    def emit_junk(pool, n, tag="warm"):
        # PE keep-warm matmuls (nothing reads the result): bridge idle
        # windows so the pstate ramp does not reset.
        jp = pool.tile([P, QC], f32, tag=tag, name=f"junk_{tag}")
        for _ in range(n):
            nc.tensor.matmul(
                jp[:], junk_sb[0:P, 0:P], junk_sb[:, 0:QC],
                start=True, stop=True,
            )

    # ---- input DMAs ----
    # ACT: weights + x qc0 (all done before the first exp); SP: ctx + x rest
    # first wk k-tile + ctx k0 quarters split so the first KT matmul starts early
    for k in range(KT_TILES):
        nc.scalar.dma_start(wq_sb[:, k, :], wq[:, k, :])
        nc.scalar.dma_start(x_sb[:, k, 0:QC], xd[:, k, 0:QC])
        nc.scalar.dma_start(wk_sb[:, k, :], wk[:, k, :])
    for k in range(KT_TILES):
        eng = nc.sync if k % 2 == 0 else nc.gpsimd
        eng.dma_start(ctx_sb[:, k, :], ctxd[:, k, :])
    nc.scalar.dma_start(wv_sb[:], wv[:])
    nc.scalar.dma_start(wo_sb[:], wo[:])
    for k in range(KT_TILES):
        nc.sync.dma_start(x_sb[:, k, QC:N], xd[:, k, QC:N])

    def emit_V(m, pool):
        vt = pool.tile([P, QC], f32, tag=pool_tag[id(pool)], name=f"vt{m}")
        for k in range(KT_TILES):
            nc.tensor.matmul(
                vt[:, 0:IC],
                ctx_sb[:, k, m * P:(m + 1) * P],
                wv_sb[:, k, :],
                start=(k == 0), stop=(k == KT_TILES - 1),
            )
        nc.vector.tensor_copy(
            v_sb[:, m, :, D:VW],
            vt[:, 0:IC].rearrange("p (h d) -> p h d", d=D),
        )

    pool_tag = {}

    # ---- PE warmup, then projections: KT (8 psum banks, k-outer), QT qc0 ----
    with tc.tile_pool(name="warm_ps", bufs=1, space="PSUM") as wps:
        emit_junk(wps, 6, tag="warm0")

    with tc.tile_pool(name="pp8", bufs=8, space="PSUM") as pps:
        pool_tag[id(pps)] = "pp"
        # QT qc0 (both j) and KT j0 interleaved per k-tile, tracking the
        # per-k weight/x/ctx DMA arrival order.
        qp = [pps.tile([P, QC], f32, tag="pp", name=f"qp{j}") for j in range(2)]
        kp0 = [pps.tile([P, QC], f32, tag="pp", name=f"kp0_{qc}") for qc in range(4)]
        for k in range(KT_TILES):
            for j in range(2):
                nc.tensor.matmul(
                    qp[j][:],
                    wq_sb[:, k, j * P:(j + 1) * P],
                    x_sb[:, k, 0:QC],
                    start=(k == 0), stop=(k == KT_TILES - 1),
                )
            for qc in range(4):
                nc.tensor.matmul(
                    kp0[qc][:],
                    wk_sb[:, k, 0:P],
                    ctx_sb[:, k, qc * QC:(qc + 1) * QC],
                    start=(k == 0), stop=(k == KT_TILES - 1),
                )
        nc.vector.tensor_copy(qt_sb[0][:, 0:QC], qp[0][:])
        nc.vector.tensor_copy(qt_sb[1][:, 0:QC], qp[1][:])
        for qc in range(4):
            nc.vector.tensor_copy(kt_sb[0][:, qc * QC:(qc + 1) * QC], kp0[qc][:])
        # KT j1 qc-major (ctx fully resident by now); last two chunks reuse
        # the QT banks, drained by the qt copies above.
        for qc in range(4):
            kp1 = pps.tile([P, QC], f32, tag="pp", name=f"kp1_{qc}")
            for k in range(KT_TILES):
                nc.tensor.matmul(
                    kp1[:],
                    wk_sb[:, k, P:2 * P],
                    ctx_sb[:, k, qc * QC:(qc + 1) * QC],
                    start=(k == 0), stop=(k == KT_TILES - 1),
                )
            nc.vector.tensor_copy(kt_sb[1][:, qc * QC:(qc + 1) * QC], kp1[:])
        # V[0]: covers the kt j1 copies + attention pool transition
        emit_V(0, pps)

    # ---- attention with interleaved fill work ----
    with ExitStack() as attn_ctx:
        sps = attn_ctx.enter_context(tc.tile_pool(name="s_ps", bufs=2, space="PSUM"))
        ops = attn_ctx.enter_context(tc.tile_pool(name="o_ps", bufs=O_BUFS, space="PSUM"))
        fps = attn_ctx.enter_context(tc.tile_pool(name="f_ps", bufs=F_BUFS, space="PSUM"))
        pool_tag[id(fps)] = "f"

        # QT fill state: one matmul per fill slot, j-serial per q-chunk
        qt_state = {}

        def emit_QT(qcn, slot):
            j, k = slot // KT_TILES, slot % KT_TILES
            if k == 0:
                qt_state[(qcn, j)] = fps.tile([P, QC], f32, tag="f", name=f"qf{qcn}_{j}")
            qtile = qt_state[(qcn, j)]
            nc.tensor.matmul(
                qtile[:],
                wq_sb[:, k, j * P:(j + 1) * P],
                x_sb[:, k, qcn * QC:(qcn + 1) * QC],
                start=(k == 0), stop=(k == KT_TILES - 1),
            )
            if k == KT_TILES - 1:
                nc.vector.tensor_copy(qt_sb[j][:, qcn * QC:(qcn + 1) * QC], qtile[:])
                del qt_state[(qcn, j)]

        # final projection fill: slot -> (nt, ec, j)
        fin_state = {}

        def emit_FIN(qcn, slot, pool):
            nt = qcn * 4 + slot // 4
            ec = (slot // 2) % 2
            j = slot % 2
            if j == 0:
                fin_state["ft"] = pool.tile(
                    [P, QC], f32, tag=pool_tag[id(pool)], name=f"ft{nt}_{ec}")
            ft = fin_state["ft"]
            nc.tensor.matmul(
                ft[:],
                ao_sb[j][:, nt * P:(nt + 1) * P],
                wo_sb[:, j, ec * QC:(ec + 1) * QC],
                start=(j == 0), stop=(j == 1),
            )
            if j == 0 and ec == 0:
                fin_state["ostg"] = outst_pool.tile([P, C], bf16, tag="ostg",
                                                    name=f"og{nt}")
            if j == 1:
                nc.vector.tensor_copy(fin_state["ostg"][:, ec * QC:(ec + 1) * QC], ft[:])
                if ec == 1:
                    deng = nc.sync if nt % 2 == 0 else nc.gpsimd
                    deng.dma_start(out[nt * P:(nt + 1) * P, :], fin_state["ostg"][:])

        # fill plan per (qc, p) segment
        def fill(qc, p, mt):
            if qc == 0 and p == 0:
                if mt + 1 < MT:
                    emit_V(mt + 1, fps)
            elif qc == 0 and p == 1:
                emit_QT(1, mt)
            elif qc == 1 and p == 0:
                emit_QT(2, mt)
            elif qc == 1 and p == 1:
                emit_FIN(0, mt, fps)
            elif qc == 2 and p == 0:
                emit_QT(3, mt)
            elif qc == 2 and p == 1:
                emit_FIN(1, mt, fps)
            elif qc == 3 and p == 0:
                emit_FIN(2, mt, fps)
            # (3,1): no fill available

        def normalize(qc, p, o_ts):
            # Chain (hh1 first; it gates the fused output projection):
            #   DVE: recip1 (straight from PSUM), oc1 copy, mul1, recip0, mul0
            #   Pool: bcast1, oc0 copy, bcast0
            # o banks freed by {recip, oc copy}; ao written by muls/pack-DMA.
            q0 = qc * QC
            rs, rbs, ocs = {}, {}, {}
            for hh in (1, 0):
                rs[hh] = r_pool.tile([P, QC], f32, tag="r", name=f"r{qc}_{p}_{hh}")
                rbs[hh] = rb_pool.tile([P, QC], f32, tag="rb", name=f"rb{qc}_{p}_{hh}")
                ocs[hh] = oc_pool.tile([P, QC], f32, tag="oc", name=f"oc{qc}_{p}_{hh}")

            def bcast(hh):
                if BCAST64:
                    nc.gpsimd.partition_broadcast(rbs[hh][0:D, :], rs[hh][64:65, :])
                else:
                    nc.sync.dma_start(rs[hh][0:1, :], rs[hh][64:65, :])
                    nc.gpsimd.partition_broadcast(rbs[hh][0:D, :], rs[hh][0:1, :])

            nc.vector.tensor_copy(ocs[1][:], o_ts[1][:])
            nc.vector.tensor_copy(ocs[0][:], o_ts[0][:])
            nc.vector.reciprocal(rs[1][0:1, :], ocs[1][0:1, :])
            nc.vector.reciprocal(rs[0][0:1, :], ocs[0][0:1, :])
            nc.gpsimd.partition_broadcast(rbs[1][0:D, :], rs[1][0:1, :])
            nc.gpsimd.partition_broadcast(rbs[0][0:D, :], rs[0][0:1, :])
            nc.gpsimd.tensor_mul(
                ao_sb[p][64:P, q0:q0 + QC], ocs[1][D:2 * D, :], rbs[1][0:D, :]
            )
            nc.gpsimd.tensor_mul(
                ao_sb[p][0:D, q0:q0 + QC], ocs[0][D:2 * D, :], rbs[0][0:D, :]
            )

        for qc in range(NQC):
            q0 = qc * QC
            for p in range(2):
                o_ts = [ops.tile([P, QC], f32, tag="o", name=f"o{qc}_{p}_{i}")
                        for i in range(2)]
                if qc == 0 and p == 0:
                    pass  # V[0], V[1] were emitted in the projection scope
                for mt in range(MT):
                    s_t = sps.tile([P, 2 * QC], f32, tag="s", name=f"s{qc}_{p}_{mt}")
                    for hh in range(2):
                        pb = hh * 64
                        nc.tensor.matmul(
                            s_t[:, hh * QC:(hh + 1) * QC],
                            kt_sb[p][pb:pb + 64, mt * P:(mt + 1) * P],
                            qt_sb[p][pb:pb + 64, q0:q0 + QC],
                            start=True, stop=True,
                        )
                    es = es_pool.tile([P, 2 * QC], bf16, tag="es", name=f"es{qc}_{p}_{mt}")
                    nc.scalar.activation(
                        es[:], s_t[:],
                        mybir.ActivationFunctionType.Exp, scale=SCALE,
                    )
                    fill(qc, p, mt)
                    for hh in range(2):
                        h = 2 * p + hh
                        nc.tensor.matmul(
                            o_ts[hh][:],
                            v_sb[:, mt, h, :],
                            es[:, hh * QC:(hh + 1) * QC],
                            start=(mt == 0), stop=(mt == MT - 1),
                        )
                normalize(qc, p, o_ts)

    # ---- tail: final projection for qc3 ----
    # ao p0 is ready well before ao p1 (its normalize ends the kernel), so:
    # j0 accumulation steps first (4 open banks), junk bridge keeps the PE
    # pstate warm while normalize(qc3,p1) completes, then the j1 steps.
    with (
        tc.tile_pool(name="tail_ps", bufs=7, space="PSUM") as tps,
        tc.tile_pool(name="tailj_ps", bufs=1, space="PSUM") as tjp,
    ):
        fts = {}
        ostgs = {}

        def fin3_mm(i, j):
            nt = 12 + i // 2
            ec = i % 2
            if j == 0:
                fts[i] = tps.tile([P, QC], f32, tag="tf", name=f"tf{i}")
            nc.tensor.matmul(
                fts[i][:],
                ao_sb[j][:, nt * P:(nt + 1) * P],
                wo_sb[:, j, ec * QC:(ec + 1) * QC],
                start=(j == 0), stop=(j == 1),
            )
            if j == 0 and ec == 0:
                ostgs[nt] = outst_pool.tile([P, C], bf16, tag="ostg", name=f"og{nt}")
            if j == 1:
                nc.vector.tensor_copy(ostgs[nt][:, ec * QC:(ec + 1) * QC], fts[i][:])
                deng = nc.sync if ec == 1 else nc.gpsimd
                deng.dma_start(out[nt * P:(nt + 1) * P, ec * QC:(ec + 1) * QC],
                               ostgs[nt][:, ec * QC:(ec + 1) * QC])

        for i in range(4):
            fin3_mm(i, 0)
        emit_junk(tjp, opt.get("tail_junk", 13), tag="warm1")
        for i in range(4):
            fin3_mm(i, 1)
        for i in range(4, 8):
            fin3_mm(i, 0)
        for i in (6, 7, 4, 5):   # last row group first so its copy+DMA drain early
            fin3_mm(i, 1)


def _build(reps=1, opt=None):
    key = (reps, tuple(sorted((opt or {}).items())))
    if key in _CACHE:
        return _CACHE[key]
    nc = bacc.Bacc("TRN2", target_bir_lowering=False, debug=False)
    xd = nc.dram_tensor("xd", [P, KT_TILES, N], bf16, kind="ExternalInput")
    ctxd = nc.dram_tensor("ctxd", [P, KT_TILES, M], bf16, kind="ExternalInput")
    wq = nc.dram_tensor("wq", [P, KT_TILES, IC], bf16, kind="ExternalInput")
    wk = nc.dram_tensor("wk", [P, KT_TILES, IC], bf16, kind="ExternalInput")
    wv = nc.dram_tensor("wv", [P, KT_TILES, IC], bf16, kind="ExternalInput")
    wo = nc.dram_tensor("wo", [P, 2, C], bf16, kind="ExternalInput")
    out = nc.dram_tensor("out", [N, C], bf16, kind="ExternalOutput")
    with tile.TileContext(nc) as tc:
        for _ in range(reps):
            with ExitStack() as ctx:
                _body(nc, tc, ctx, xd, ctxd, wq, wk, wv, wo, out, opt=opt)
    nc.compile()
    _CACHE[key] = nc
    return nc


def _to_tiled(a, inner):
    """[K*128, inner] f32 -> [128, K, inner] bf16 (partition-major tiling)."""
    k = a.shape[0] // P
    return np.ascontiguousarray(
        a.reshape(k, P, inner).transpose(1, 0, 2).astype(ml_dtypes.bfloat16)
    )


def _shard_inputs(x, context, Wq, Wk, Wv, Wo):
    in_maps = []
    for c in range(NCORES):
        b, g = divmod(c, NCORES // B)
        cols = slice(g * IC, (g + 1) * IC)
        in_maps.append({
            "xd": _to_tiled(np.ascontiguousarray(x[b].T), N),
            "ctxd": _to_tiled(np.ascontiguousarray(context[b].T), M),
            "wq": _to_tiled(np.ascontiguousarray(Wq[:, cols]), IC),
            "wk": _to_tiled(np.ascontiguousarray(Wk[:, cols]), IC),
            "wv": _to_tiled(np.ascontiguousarray(Wv[:, cols]), IC),
            "wo": _to_tiled(np.ascontiguousarray(Wo[cols, :]), C),
        })
    return in_maps


def kernel(x, context, Wq, Wk, Wv, Wo, reps=1):
    x = np.asarray(x, dtype=np.float32)
    context = np.asarray(context, dtype=np.float32)
    Wq, Wk, Wv, Wo = (np.asarray(w, dtype=np.float32) for w in (Wq, Wk, Wv, Wo))
    nc = _build(reps)
    in_maps = _shard_inputs(x, context, Wq, Wk, Wv, Wo)
    res = run_bass_kernel_spmd(nc, in_maps, core_ids=list(range(NCORES)))
    gpb = NCORES // B
    out = np.zeros((B, N, C), dtype=np.float32)
    for c in range(NCORES):
        out[c // gpb] += np.asarray(res.results[c]["out"], dtype=np.float32)
    return out


# revision 33
# speedup vs baseline: 1.0397x; 1.0100x over previous
"""Trainium2 Bass kernel for CrossAttention (B=2, N=M=2048, 16 heads x 64).

Sharding: batch x head-group parallel over 8 cores. Core c handles batch
c//4 and heads [4*(c%4), 4*(c%4)+4). Projection weights are column-split
(Wq/Wk/Wv) / row-split (Wo) per core; each core produces a partial
[2048, 1024] output (bf16) which the host sums per batch (4 partials).

V2 design (cost-model driven):
  - All DRAM inputs and SBUF matmul operands are bf16 (same 1 cycle/row
    PE speed as f32r, half the DMA bytes, ~5e-3 rel err total).
  - ACT engine runs ONLY the softmax Exp (131072 rows = the 133us floor);
    all DMAs/copies live on SP/DVE/Pool queues.
  - PE is the bottleneck (393216 matmul rows = 163.8us floor).  Emission
    keeps PE saturated: KT -> QT(qc0) -> per q-chunk/head-pair S/exp/O
    streams with "fill" matmuls (QT for later q-chunks, fused output
    projection of earlier q-chunks, V during the first chunk) interleaved
    one per m-tile to absorb the exp-vs-PE rate gap.
  - Normalization: ones column in V_aug yields denominators in PSUM row
    64; DVE reciprocal, SP DMA to partition 0, gpsimd partition_broadcast,
    DVE multiply (bf16 out).  PSUM: 2x s_t (2 banks each) + 3x o_t +
    1 fill bank = 8 banks.
"""

import numpy as np
import ml_dtypes
from contextlib import ExitStack

import concourse.tile as tile
from concourse import bacc, mybir
from concourse.bass_utils import run_bass_kernel_spmd

B, N, M, C = 2, 2048, 2048, 1024
HEADS, D = 16, 64
HPC = 4            # heads per core
IC = HPC * D       # 256 inner dims per core
SCALE = D ** -0.5
NCORES = 8
KT_TILES = C // 128   # 8 contraction tiles for projections
P = 128
MT = M // P           # 16 m tiles
QC = 512
NQC = N // QC         # 4 q chunks
f32 = mybir.dt.float32
bf16 = mybir.dt.bfloat16

_CACHE = {}


def _body(nc, tc, ctx, xd, ctxd, wq, wk, wv, wo, out, opt=None):
    opt = opt or {}
    ES_BUFS = opt.get("es_bufs", 4)
    O_BUFS = opt.get("o_bufs", 2)
    F_BUFS = opt.get("f_bufs", 2)
    BCAST64 = opt.get("bcast64", False)
    GP_AOPACK = opt.get("gp_aopack", True)
    ONES0 = opt.get("ones0", True)

    const = ctx.enter_context(tc.tile_pool(name="const", bufs=1))
    wq_sb = const.tile([P, KT_TILES, IC], bf16, tag="wq")
    wk_sb = const.tile([P, KT_TILES, IC], bf16, tag="wk")
    wv_sb = const.tile([P, KT_TILES, IC], bf16, tag="wv")
    wo_sb = const.tile([P, 2, C], bf16, tag="wo")
    ctx_sb = const.tile([P, KT_TILES, M], bf16, tag="ctx")
    x_sb = const.tile([P, KT_TILES, N], bf16, tag="x")
    kt_sb = [const.tile([P, M], bf16, tag=f"kt{j}", name=f"kt{j}") for j in range(2)]
    qt_sb = [const.tile([P, N], bf16, tag=f"qt{j}", name=f"qt{j}") for j in range(2)]
    VW = 2 * D  # ones | zeros pad | V values
    v_sb = const.tile([P, MT, HPC, VW], bf16, tag="v")
    ao_sb = [const.tile([P, N], bf16, tag=f"ao{j}", name=f"ao{j}") for j in range(2)]
    ones_sb = const.tile([P, 1], f32, tag="ones")

    es_pool = ctx.enter_context(tc.tile_pool(name="es", bufs=ES_BUFS))
    r_pool = ctx.enter_context(tc.tile_pool(name="rp", bufs=2))
    rb_pool = ctx.enter_context(tc.tile_pool(name="rbp", bufs=2))
    oc_pool = ctx.enter_context(tc.tile_pool(name="ocp", bufs=4))
    aot_pool = ctx.enter_context(tc.tile_pool(name="aot", bufs=2))
    outst_pool = ctx.enter_context(tc.tile_pool(name="outst", bufs=4))

    junk_sb = const.tile([P, QC], bf16, tag="junk")

    # junk tile first on DVE so PE warmup matmuls can start ~1us in
    nc.vector.memset(junk_sb[:], 0.0)
    # ones column of V_aug (f32 memset + broadcast-copy cast to bf16)
    nc.vector.memset(ones_sb[:], 1.0)
    nc.vector.memset(v_sb[:, :, :, 1:D], 0.0)
    nc.vector.tensor_copy(
        v_sb[:, :, :, 0:1],
        ones_sb[:, 0:1].to_broadcast((P, MT, HPC, 1)),
    )

    def emit_junk(pool, n, tag="warm"):
        # PE keep-warm matmuls (nothing reads the result): bridge idle
        # windows so the pstate ramp does not reset.
        jp = pool.tile([P, QC], f32, tag=tag, name=f"junk_{tag}")
        for _ in range(n):
            nc.tensor.matmul(
                jp[:], junk_sb[0:P, 0:P], junk_sb[:, 0:QC],
                start=True, stop=True,
            )

    # ---- input DMAs ----
    # ACT: weights + x qc0 (all done before the first exp); SP: ctx + x rest
    # first wk k-tile + ctx k0 quarters split so the first KT matmul starts early
    for k in range(KT_TILES):
        nc.scalar.dma_start(wq_sb[:, k, :], wq[:, k, :])
        nc.scalar.dma_start(x_sb[:, k, 0:QC], xd[:, k, 0:QC])
        nc.scalar.dma_start(wk_sb[:, k, :], wk[:, k, :])
    for k in range(KT_TILES):
        eng = nc.sync if k % 2 == 0 else nc.gpsimd
        eng.dma_start(ctx_sb[:, k, :], ctxd[:, k, :])
    nc.scalar.dma_start(wv_sb[:], wv[:])
    nc.scalar.dma_start(wo_sb[:], wo[:])
    for k in range(KT_TILES):
        nc.sync.dma_start(x_sb[:, k, QC:N], xd[:, k, QC:N])

    def emit_V(m, pool):
        vt = pool.tile([P, QC], f32, tag=pool_tag[id(pool)], name=f"vt{m}")
        for k in range(KT_TILES):
            nc.tensor.matmul(
                vt[:, 0:IC],
                ctx_sb[:, k, m * P:(m + 1) * P],
                wv_sb[:, k, :],
                start=(k == 0), stop=(k == KT_TILES - 1),
            )
        nc.vector.tensor_copy(
            v_sb[:, m, :, D:VW],
            vt[:, 0:IC].rearrange("p (h d) -> p h d", d=D),
        )

    pool_tag = {}

    # ---- PE warmup, then projections: KT (8 psum banks, k-outer), QT qc0 ----
    with tc.tile_pool(name="warm_ps", bufs=1, space="PSUM") as wps:
        emit_junk(wps, 6, tag="warm0")

    with tc.tile_pool(name="pp8", bufs=8, space="PSUM") as pps:
        pool_tag[id(pps)] = "pp"
        # QT qc0 (both j) and KT j0 interleaved per k-tile, tracking the
        # per-k weight/x/ctx DMA arrival order.
        qp = [pps.tile([P, QC], f32, tag="pp", name=f"qp{j}") for j in range(2)]
        kp0 = [pps.tile([P, QC], f32, tag="pp", name=f"kp0_{qc}") for qc in range(4)]
        for k in range(KT_TILES):
            for j in range(2):
                nc.tensor.matmul(
                    qp[j][:],
                    wq_sb[:, k, j * P:(j + 1) * P],
                    x_sb[:, k, 0:QC],
                    start=(k == 0), stop=(k == KT_TILES - 1),
                )
            for qc in range(4):
                nc.tensor.matmul(
                    kp0[qc][:],
                    wk_sb[:, k, 0:P],
                    ctx_sb[:, k, qc * QC:(qc + 1) * QC],
                    start=(k == 0), stop=(k == KT_TILES - 1),
                )
        nc.vector.tensor_copy(qt_sb[0][:, 0:QC], qp[0][:])
        nc.vector.tensor_copy(qt_sb[1][:, 0:QC], qp[1][:])
        for qc in range(4):
            nc.vector.tensor_copy(kt_sb[0][:, qc * QC:(qc + 1) * QC], kp0[qc][:])
        # KT j1 qc-major (ctx fully resident by now); last two chunks reuse
        # the QT banks, drained by the qt copies above.
        for qc in range(4):
            kp1 = pps.tile([P, QC], f32, tag="pp", name=f"kp1_{qc}")
            for k in range(KT_TILES):
                nc.tensor.matmul(
                    kp1[:],
                    wk_sb[:, k, P:2 * P],
                    ctx_sb[:, k, qc * QC:(qc + 1) * QC],
                    start=(k == 0), stop=(k == KT_TILES - 1),
                )
            nc.vector.tensor_copy(kt_sb[1][:, qc * QC:(qc + 1) * QC], kp1[:])
        # V[0]: covers the kt j1 copies + attention pool transition
        emit_V(0, pps)

    # ---- attention with interleaved fill work ----
    with ExitStack() as attn_ctx:
        sps = attn_ctx.enter_context(tc.tile_pool(name="s_ps", bufs=2, space="PSUM"))
        ops = attn_ctx.enter_context(tc.tile_pool(name="o_ps", bufs=O_BUFS, space="PSUM"))
        fps = attn_ctx.enter_context(tc.tile_pool(name="f_ps", bufs=F_BUFS, space="PSUM"))
        pool_tag[id(fps)] = "f"

        # QT fill state: one matmul per fill slot, j-serial per q-chunk
        qt_state = {}

        def emit_QT(qcn, slot):
            j, k = slot // KT_TILES, slot % KT_TILES
            if k == 0:
                qt_state[(qcn, j)] = fps.tile([P, QC], f32, tag="f", name=f"qf{qcn}_{j}")
            qtile = qt_state[(qcn, j)]
            nc.tensor.matmul(
                qtile[:],
                wq_sb[:, k, j * P:(j + 1) * P],
                x_sb[:, k, qcn * QC:(qcn + 1) * QC],
                start=(k == 0), stop=(k == KT_TILES - 1),
            )
            if k == KT_TILES - 1:
                nc.vector.tensor_copy(qt_sb[j][:, qcn * QC:(qcn + 1) * QC], qtile[:])
                del qt_state[(qcn, j)]

        # final projection fill: slot -> (nt, ec, j)
        fin_state = {}

        def emit_FIN(qcn, slot, pool):
            nt = qcn * 4 + slot // 4
            ec = (slot // 2) % 2
            j = slot % 2
            if j == 0:
                fin_state["ft"] = pool.tile(
                    [P, QC], f32, tag=pool_tag[id(pool)], name=f"ft{nt}_{ec}")
            ft = fin_state["ft"]
            nc.tensor.matmul(
                ft[:],
                ao_sb[j][:, nt * P:(nt + 1) * P],
                wo_sb[:, j, ec * QC:(ec + 1) * QC],
                start=(j == 0), stop=(j == 1),
            )
            if j == 0 and ec == 0:
                fin_state["ostg"] = outst_pool.tile([P, C], bf16, tag="ostg",
                                                    name=f"og{nt}")
            if j == 1:
                nc.vector.tensor_copy(fin_state["ostg"][:, ec * QC:(ec + 1) * QC], ft[:])
                if ec == 1:
                    deng = nc.sync if nt % 2 == 0 else nc.gpsimd
                    deng.dma_start(out[nt * P:(nt + 1) * P, :], fin_state["ostg"][:])

        # fill plan per (qc, p) segment
        def fill(qc, p, mt):
            if qc == 0 and p == 0:
                if mt + 1 < MT:
                    emit_V(mt + 1, fps)
            elif qc == 0 and p == 1:
                emit_QT(1, mt)
            elif qc == 1 and p == 0:
                emit_QT(2, mt)
            elif qc == 1 and p == 1:
                emit_FIN(0, mt, fps)
            elif qc == 2 and p == 0:
                emit_QT(3, mt)
            elif qc == 2 and p == 1:
                emit_FIN(1, mt, fps)
            elif qc == 3 and p == 0:
                emit_FIN(2, mt, fps)
            # (3,1): no fill available

        def normalize(qc, p, o_ts):
            # Chain (hh1 first; it gates the fused output projection):
            #   DVE: recip1 (straight from PSUM), oc1 copy, mul1, recip0, mul0
            #   Pool: bcast1, oc0 copy, bcast0
            # o banks freed by {recip, oc copy}; ao written by muls/pack-DMA.
            q0 = qc * QC
            rs, rbs, ocs = {}, {}, {}
            for hh in (1, 0):
                rs[hh] = r_pool.tile([P, QC], f32, tag="r", name=f"r{qc}_{p}_{hh}")
                rbs[hh] = rb_pool.tile([P, QC], f32, tag="rb", name=f"rb{qc}_{p}_{hh}")
                ocs[hh] = oc_pool.tile([P, QC], f32, tag="oc", name=f"oc{qc}_{p}_{hh}")

            def bcast(hh):
                if BCAST64:
                    nc.gpsimd.partition_broadcast(rbs[hh][0:D, :], rs[hh][64:65, :])
                else:
                    nc.sync.dma_start(rs[hh][0:1, :], rs[hh][64:65, :])
                    nc.gpsimd.partition_broadcast(rbs[hh][0:D, :], rs[hh][0:1, :])

            nc.vector.tensor_copy(ocs[1][:], o_ts[1][:])
            nc.vector.tensor_copy(ocs[0][:], o_ts[0][:])
            nc.vector.reciprocal(rs[1][0:1, :], ocs[1][0:1, :])
            nc.vector.reciprocal(rs[0][0:1, :], ocs[0][0:1, :])
            nc.gpsimd.partition_broadcast(rbs[1][0:D, :], rs[1][0:1, :])
            nc.gpsimd.partition_broadcast(rbs[0][0:D, :], rs[0][0:1, :])
            nc.gpsimd.tensor_mul(
                ao_sb[p][64:P, q0:q0 + QC], ocs[1][D:2 * D, :], rbs[1][0:D, :]
            )
            nc.gpsimd.tensor_mul(
                ao_sb[p][0:D, q0:q0 + QC], ocs[0][D:2 * D, :], rbs[0][0:D, :]
            )

        for qc in range(NQC):
            q0 = qc * QC
            for p in range(2):
                o_ts = [ops.tile([P, QC], f32, tag="o", name=f"o{qc}_{p}_{i}")
                        for i in range(2)]
                if qc == 0 and p == 0:
                    pass  # V[0], V[1] were emitted in the projection scope
                for mt in range(MT):
                    s_t = sps.tile([P, 2 * QC], f32, tag="s", name=f"s{qc}_{p}_{mt}")
                    for hh in range(2):
                        pb = hh * 64
                        nc.tensor.matmul(
                            s_t[:, hh * QC:(hh + 1) * QC],
                            kt_sb[p][pb:pb + 64, mt * P:(mt + 1) * P],
                            qt_sb[p][pb:pb + 64, q0:q0 + QC],
                            start=True, stop=True,
                        )
                    es = es_pool.tile([P, 2 * QC], bf16, tag="es", name=f"es{qc}_{p}_{mt}")
                    nc.scalar.activation(
                        es[:], s_t[:],
                        mybir.ActivationFunctionType.Exp, scale=SCALE,
                    )
                    fill(qc, p, mt)
                    for hh in range(2):
                        h = 2 * p + hh
                        nc.tensor.matmul(
                            o_ts[hh][:],
                            v_sb[:, mt, h, :],
                            es[:, hh * QC:(hh + 1) * QC],
                            start=(mt == 0), stop=(mt == MT - 1),
                        )
                normalize(qc, p, o_ts)

    # ---- tail: final projection for qc3 ----
    # ao p0 is ready well before ao p1 (its normalize ends the kernel), so:
    # j0 accumulation steps first (4 open banks), junk bridge keeps the PE
    # pstate warm while normalize(qc3,p1) completes, then the j1 steps.
    with (
        tc.tile_pool(name="tail_ps", bufs=7, space="PSUM") as tps,
        tc.tile_pool(name="tailj_ps", bufs=1, space="PSUM") as tjp,
    ):
        fts = {}
        ostgs = {}

        def fin3_mm(i, j):
            nt = 12 + i // 2
            ec = i % 2
            if j == 0:
                fts[i] = tps.tile([P, QC], f32, tag="tf", name=f"tf{i}")
            nc.tensor.matmul(
                fts[i][:],
                ao_sb[j][:, nt * P:(nt + 1) * P],
                wo_sb[:, j, ec * QC:(ec + 1) * QC],
                start=(j == 0), stop=(j == 1),
            )
            if j == 0 and ec == 0:
                ostgs[nt] = outst_pool.tile([P, C], bf16, tag="ostg", name=f"og{nt}")
            if j == 1:
                nc.vector.tensor_copy(ostgs[nt][:, ec * QC:(ec + 1) * QC], fts[i][:])
                deng = nc.sync if ec == 1 else nc.gpsimd
                deng.dma_start(out[nt * P:(nt + 1) * P, ec * QC:(ec + 1) * QC],
                               ostgs[nt][:, ec * QC:(ec + 1) * QC])

        for i in range(4):
            fin3_mm(i, 0)
        emit_junk(tjp, opt.get("tail_junk", 13), tag="warm1")
        for i in range(4):
            fin3_mm(i, 1)
        for i in range(4, 8):
            fin3_mm(i, 0)
        for i in (6, 7, 4, 5):   # last row group first so its copy+DMA drain early
            fin3_mm(i, 1)


def _build(reps=1, opt=None):
    key = (reps, tuple(sorted((opt or {}).items())))
    if key in _CACHE:
        return _CACHE[key]
    nc = bacc.Bacc("TRN2", target_bir_lowering=False, debug=False)
    xd = nc.dram_tensor("xd", [P, KT_TILES, N], bf16, kind="ExternalInput")
    ctxd = nc.dram_tensor("ctxd", [P, KT_TILES, M], bf16, kind="ExternalInput")
    wq = nc.dram_tensor("wq", [P, KT_TILES, IC], bf16, kind="ExternalInput")
    wk = nc.dram_tensor("wk", [P, KT_TILES, IC], bf16, kind="ExternalInput")
    wv = nc.dram_tensor("wv", [P, KT_TILES, IC], bf16, kind="ExternalInput")
    wo = nc.dram_tensor("wo", [P, 2, C], bf16, kind="ExternalInput")
    out = nc.dram_tensor("out", [N, C], bf16, kind="ExternalOutput")
    with tile.TileContext(nc) as tc:
        for _ in range(reps):
            with ExitStack() as ctx:
                _body(nc, tc, ctx, xd, ctxd, wq, wk, wv, wo, out, opt=opt)
    nc.compile()
    _CACHE[key] = nc
    return nc


def _to_tiled(a, inner):
    """[K*128, inner] f32 -> [128, K, inner] bf16 (partition-major tiling)."""
    k = a.shape[0] // P
    return np.ascontiguousarray(
        a.reshape(k, P, inner).transpose(1, 0, 2).astype(ml_dtypes.bfloat16)
    )


def _shard_inputs(x, context, Wq, Wk, Wv, Wo):
    in_maps = []
    for c in range(NCORES):
        b, g = divmod(c, NCORES // B)
        cols = slice(g * IC, (g + 1) * IC)
        in_maps.append({
            "xd": _to_tiled(np.ascontiguousarray(x[b].T), N),
            "ctxd": _to_tiled(np.ascontiguousarray(context[b].T), M),
            "wq": _to_tiled(np.ascontiguousarray(Wq[:, cols]), IC),
            "wk": _to_tiled(np.ascontiguousarray(Wk[:, cols]), IC),
            "wv": _to_tiled(np.ascontiguousarray(Wv[:, cols]), IC),
            "wo": _to_tiled(np.ascontiguousarray(Wo[cols, :]), C),
        })
    return in_maps


def kernel(x, context, Wq, Wk, Wv, Wo, reps=1):
    x = np.asarray(x, dtype=np.float32)
    context = np.asarray(context, dtype=np.float32)
    Wq, Wk, Wv, Wo = (np.asarray(w, dtype=np.float32) for w in (Wq, Wk, Wv, Wo))
    nc = _build(reps)
    in_maps = _shard_inputs(x, context, Wq, Wk, Wv, Wo)
    res = run_bass_kernel_spmd(nc, in_maps, core_ids=list(range(NCORES)))
    gpb = NCORES // B
    out = np.zeros((B, N, C), dtype=np.float32)
    for c in range(NCORES):
        out[c // gpb] += np.asarray(res.results[c]["out"], dtype=np.float32)
    return out


# revision 34
# speedup vs baseline: 1.0431x; 1.0033x over previous
"""Trainium2 Bass kernel for CrossAttention (B=2, N=M=2048, 16 heads x 64).

Sharding: batch x head-group parallel over 8 cores. Core c handles batch
c//4 and heads [4*(c%4), 4*(c%4)+4). Projection weights are column-split
(Wq/Wk/Wv) / row-split (Wo) per core; each core produces a partial
[2048, 1024] output (bf16) which the host sums per batch (4 partials).

V2 design (cost-model driven):
  - All DRAM inputs and SBUF matmul operands are bf16 (same 1 cycle/row
    PE speed as f32r, half the DMA bytes, ~5e-3 rel err total).
  - ACT engine runs ONLY the softmax Exp (131072 rows = the 133us floor);
    all DMAs/copies live on SP/DVE/Pool queues.
  - PE is the bottleneck (393216 matmul rows = 163.8us floor).  Emission
    keeps PE saturated: KT -> QT(qc0) -> per q-chunk/head-pair S/exp/O
    streams with "fill" matmuls (QT for later q-chunks, fused output
    projection of earlier q-chunks, V during the first chunk) interleaved
    one per m-tile to absorb the exp-vs-PE rate gap.
  - Normalization: ones column in V_aug yields denominators in PSUM row
    64; DVE reciprocal, SP DMA to partition 0, gpsimd partition_broadcast,
    DVE multiply (bf16 out).  PSUM: 2x s_t (2 banks each) + 3x o_t +
    1 fill bank = 8 banks.
"""

import numpy as np
import ml_dtypes
from contextlib import ExitStack

import concourse.tile as tile
from concourse import bacc, mybir
from concourse.bass_utils import run_bass_kernel_spmd

B, N, M, C = 2, 2048, 2048, 1024
HEADS, D = 16, 64
HPC = 4            # heads per core
IC = HPC * D       # 256 inner dims per core
SCALE = D ** -0.5
NCORES = 8
KT_TILES = C // 128   # 8 contraction tiles for projections
P = 128
MT = M // P           # 16 m tiles
QC = 512
NQC = N // QC         # 4 q chunks
f32 = mybir.dt.float32
bf16 = mybir.dt.bfloat16

_CACHE = {}


def _body(nc, tc, ctx, xd, ctxd, wq, wk, wv, wo, out, opt=None):
    opt = opt or {}
    ES_BUFS = opt.get("es_bufs", 4)
    O_BUFS = opt.get("o_bufs", 2)
    F_BUFS = opt.get("f_bufs", 2)
    BCAST64 = opt.get("bcast64", False)
    GP_AOPACK = opt.get("gp_aopack", True)
    ONES0 = opt.get("ones0", True)

    const = ctx.enter_context(tc.tile_pool(name="const", bufs=1))
    wq_sb = const.tile([P, KT_TILES, IC], bf16, tag="wq")
    wk_sb = const.tile([P, KT_TILES, IC], bf16, tag="wk")
    wv_sb = const.tile([P, KT_TILES, IC], bf16, tag="wv")
    wo_sb = const.tile([P, 2, C], bf16, tag="wo")
    ctx_sb = const.tile([P, KT_TILES, M], bf16, tag="ctx")
    x_sb = const.tile([P, KT_TILES, N], bf16, tag="x")
    kt_sb = [const.tile([P, M], bf16, tag=f"kt{j}", name=f"kt{j}") for j in range(2)]
    qt_sb = [const.tile([P, N], bf16, tag=f"qt{j}", name=f"qt{j}") for j in range(2)]
    VW = 2 * D  # ones | zeros pad | V values
    v_sb = const.tile([P, MT, HPC, VW], bf16, tag="v")
    ao_sb = [const.tile([P, N], bf16, tag=f"ao{j}", name=f"ao{j}") for j in range(2)]
    ones_sb = const.tile([P, 1], f32, tag="ones")

    es_pool = ctx.enter_context(tc.tile_pool(name="es", bufs=ES_BUFS))
    r_pool = ctx.enter_context(tc.tile_pool(name="rp", bufs=2))
    rb_pool = ctx.enter_context(tc.tile_pool(name="rbp", bufs=2))
    oc_pool = ctx.enter_context(tc.tile_pool(name="ocp", bufs=4))
    aot_pool = ctx.enter_context(tc.tile_pool(name="aot", bufs=2))
    outst_pool = ctx.enter_context(tc.tile_pool(name="outst", bufs=4))

    junk_sb = const.tile([P, QC], bf16, tag="junk")

    # junk tile first on DVE so PE warmup matmuls can start ~1us in
    nc.vector.memset(junk_sb[:], 0.0)
    # ones column of V_aug (f32 memset + broadcast-copy cast to bf16)
    nc.vector.memset(ones_sb[:], 1.0)
    nc.vector.memset(v_sb[:, :, :, 1:D], 0.0)
    nc.vector.tensor_copy(
        v_sb[:, :, :, 0:1],
        ones_sb[:, 0:1].to_broadcast((P, MT, HPC, 1)),
    )

    def emit_junk(pool, n, tag="warm"):
        # PE keep-warm matmuls (nothing reads the result): bridge idle
        # windows so the pstate ramp does not reset.
        jp = pool.tile([P, QC], f32, tag=tag, name=f"junk_{tag}")
        for _ in range(n):
            nc.tensor.matmul(
                jp[:], junk_sb[0:P, 0:P], junk_sb[:, 0:QC],
                start=True, stop=True,
            )

    # ---- input DMAs ----
    # ACT: weights + x qc0 (all done before the first exp); SP: ctx + x rest
    # first wk k-tile + ctx k0 quarters split so the first KT matmul starts early
    for k in range(KT_TILES):
        nc.scalar.dma_start(wq_sb[:, k, :], wq[:, k, :])
        nc.scalar.dma_start(x_sb[:, k, 0:QC], xd[:, k, 0:QC])
        nc.scalar.dma_start(wk_sb[:, k, :], wk[:, k, :])
    for k in range(KT_TILES):
        eng = nc.sync if k % 2 == 0 else nc.gpsimd
        eng.dma_start(ctx_sb[:, k, :], ctxd[:, k, :])
    nc.scalar.dma_start(wv_sb[:], wv[:])
    nc.scalar.dma_start(wo_sb[:], wo[:])
    for k in range(KT_TILES):
        nc.sync.dma_start(x_sb[:, k, QC:N], xd[:, k, QC:N])

    def emit_V(m, pool):
        vt = pool.tile([P, QC], f32, tag=pool_tag[id(pool)], name=f"vt{m}")
        for k in range(KT_TILES):
            nc.tensor.matmul(
                vt[:, 0:IC],
                ctx_sb[:, k, m * P:(m + 1) * P],
                wv_sb[:, k, :],
                start=(k == 0), stop=(k == KT_TILES - 1),
            )
        nc.vector.tensor_copy(
            v_sb[:, m, :, D:VW],
            vt[:, 0:IC].rearrange("p (h d) -> p h d", d=D),
        )

    pool_tag = {}

    # ---- PE warmup, then projections: KT (8 psum banks, k-outer), QT qc0 ----
    with tc.tile_pool(name="warm_ps", bufs=1, space="PSUM") as wps:
        emit_junk(wps, 6, tag="warm0")

    with tc.tile_pool(name="pp8", bufs=8, space="PSUM") as pps:
        pool_tag[id(pps)] = "pp"
        # QT qc0 (both j) and KT j0 interleaved per k-tile, tracking the
        # per-k weight/x/ctx DMA arrival order.
        qp = [pps.tile([P, QC], f32, tag="pp", name=f"qp{j}") for j in range(2)]
        kp0 = [pps.tile([P, QC], f32, tag="pp", name=f"kp0_{qc}") for qc in range(4)]
        for k in range(KT_TILES):
            for j in range(2):
                nc.tensor.matmul(
                    qp[j][:],
                    wq_sb[:, k, j * P:(j + 1) * P],
                    x_sb[:, k, 0:QC],
                    start=(k == 0), stop=(k == KT_TILES - 1),
                )
            for qc in range(4):
                nc.tensor.matmul(
                    kp0[qc][:],
                    wk_sb[:, k, 0:P],
                    ctx_sb[:, k, qc * QC:(qc + 1) * QC],
                    start=(k == 0), stop=(k == KT_TILES - 1),
                )
        nc.vector.tensor_copy(qt_sb[0][:, 0:QC], qp[0][:])
        nc.vector.tensor_copy(qt_sb[1][:, 0:QC], qp[1][:])
        for qc in range(4):
            nc.vector.tensor_copy(kt_sb[0][:, qc * QC:(qc + 1) * QC], kp0[qc][:])
        # KT j1 qc-major (ctx fully resident by now); last two chunks reuse
        # the QT banks, drained by the qt copies above.
        for qc in range(4):
            kp1 = pps.tile([P, QC], f32, tag="pp", name=f"kp1_{qc}")
            for k in range(KT_TILES):
                nc.tensor.matmul(
                    kp1[:],
                    wk_sb[:, k, P:2 * P],
                    ctx_sb[:, k, qc * QC:(qc + 1) * QC],
                    start=(k == 0), stop=(k == KT_TILES - 1),
                )
            nc.vector.tensor_copy(kt_sb[1][:, qc * QC:(qc + 1) * QC], kp1[:])
        # V[0]: covers the kt j1 copies + attention pool transition
        emit_V(0, pps)

    # ---- attention with interleaved fill work ----
    with ExitStack() as attn_ctx:
        sps = attn_ctx.enter_context(tc.tile_pool(name="s_ps", bufs=2, space="PSUM"))
        ops = attn_ctx.enter_context(tc.tile_pool(name="o_ps", bufs=O_BUFS, space="PSUM"))
        fps = attn_ctx.enter_context(tc.tile_pool(name="f_ps", bufs=F_BUFS, space="PSUM"))
        pool_tag[id(fps)] = "f"

        # QT fill state: one matmul per fill slot, j-serial per q-chunk
        qt_state = {}

        def emit_QT(qcn, slot):
            j, k = slot // KT_TILES, slot % KT_TILES
            if k == 0:
                qt_state[(qcn, j)] = fps.tile([P, QC], f32, tag="f", name=f"qf{qcn}_{j}")
            qtile = qt_state[(qcn, j)]
            nc.tensor.matmul(
                qtile[:],
                wq_sb[:, k, j * P:(j + 1) * P],
                x_sb[:, k, qcn * QC:(qcn + 1) * QC],
                start=(k == 0), stop=(k == KT_TILES - 1),
            )
            if k == KT_TILES - 1:
                nc.vector.tensor_copy(qt_sb[j][:, qcn * QC:(qcn + 1) * QC], qtile[:])
                del qt_state[(qcn, j)]

        # final projection fill: slot -> (nt, ec, j)
        fin_state = {}

        def emit_FIN(qcn, slot, pool):
            nt = qcn * 4 + slot // 4
            ec = (slot // 2) % 2
            j = slot % 2
            if j == 0:
                fin_state["ft"] = pool.tile(
                    [P, QC], f32, tag=pool_tag[id(pool)], name=f"ft{nt}_{ec}")
            ft = fin_state["ft"]
            nc.tensor.matmul(
                ft[:],
                ao_sb[j][:, nt * P:(nt + 1) * P],
                wo_sb[:, j, ec * QC:(ec + 1) * QC],
                start=(j == 0), stop=(j == 1),
            )
            if j == 0 and ec == 0:
                fin_state["ostg"] = outst_pool.tile([P, C], bf16, tag="ostg",
                                                    name=f"og{nt}")
            if j == 1:
                nc.vector.tensor_copy(fin_state["ostg"][:, ec * QC:(ec + 1) * QC], ft[:])
                if ec == 1:
                    deng = nc.sync if nt % 2 == 0 else nc.gpsimd
                    deng.dma_start(out[nt * P:(nt + 1) * P, :], fin_state["ostg"][:])

        # fill plan per (qc, p) segment
        def fill(qc, p, mt):
            if qc == 0 and p == 0:
                if mt + 1 < MT:
                    emit_V(mt + 1, fps)
            elif qc == 0 and p == 1:
                emit_QT(1, mt)
            elif qc == 1 and p == 0:
                emit_QT(2, mt)
            elif qc == 1 and p == 1:
                emit_FIN(0, mt, fps)
            elif qc == 2 and p == 0:
                emit_QT(3, mt)
            elif qc == 2 and p == 1:
                emit_FIN(1, mt, fps)
            elif qc == 3 and p == 0:
                emit_FIN(2, mt, fps)
            # (3,1): no fill available

        def normalize(qc, p, o_ts):
            # Chain (hh1 first; it gates the fused output projection):
            #   DVE: recip1 (straight from PSUM), oc1 copy, mul1, recip0, mul0
            #   Pool: bcast1, oc0 copy, bcast0
            # o banks freed by {recip, oc copy}; ao written by muls/pack-DMA.
            q0 = qc * QC
            rs, rbs, ocs = {}, {}, {}
            for hh in (1, 0):
                rs[hh] = r_pool.tile([P, QC], f32, tag="r", name=f"r{qc}_{p}_{hh}")
                rbs[hh] = rb_pool.tile([P, QC], f32, tag="rb", name=f"rb{qc}_{p}_{hh}")
                ocs[hh] = oc_pool.tile([P, QC], f32, tag="oc", name=f"oc{qc}_{p}_{hh}")

            def bcast(hh):
                if BCAST64:
                    nc.gpsimd.partition_broadcast(rbs[hh][0:D, :], rs[hh][64:65, :])
                else:
                    nc.sync.dma_start(rs[hh][0:1, :], rs[hh][64:65, :])
                    nc.gpsimd.partition_broadcast(rbs[hh][0:D, :], rs[hh][0:1, :])

            if qc == NQC - 1 and p == 1:
                nc.scalar.copy(ocs[1][:], o_ts[1][:])
                nc.scalar.copy(ocs[0][:], o_ts[0][:])
            else:
                nc.vector.tensor_copy(ocs[1][:], o_ts[1][:])
                nc.vector.tensor_copy(ocs[0][:], o_ts[0][:])
            nc.vector.reciprocal(rs[1][0:1, :], ocs[1][0:1, :])
            nc.vector.reciprocal(rs[0][0:1, :], ocs[0][0:1, :])
            nc.gpsimd.partition_broadcast(rbs[1][0:D, :], rs[1][0:1, :])
            nc.gpsimd.partition_broadcast(rbs[0][0:D, :], rs[0][0:1, :])
            nc.gpsimd.tensor_mul(
                ao_sb[p][64:P, q0:q0 + QC], ocs[1][D:2 * D, :], rbs[1][0:D, :]
            )
            nc.gpsimd.tensor_mul(
                ao_sb[p][0:D, q0:q0 + QC], ocs[0][D:2 * D, :], rbs[0][0:D, :]
            )

        for qc in range(NQC):
            q0 = qc * QC
            for p in range(2):
                o_ts = [ops.tile([P, QC], f32, tag="o", name=f"o{qc}_{p}_{i}")
                        for i in range(2)]
                if qc == 0 and p == 0:
                    pass  # V[0], V[1] were emitted in the projection scope
                for mt in range(MT):
                    s_t = sps.tile([P, 2 * QC], f32, tag="s", name=f"s{qc}_{p}_{mt}")
                    for hh in range(2):
                        pb = hh * 64
                        nc.tensor.matmul(
                            s_t[:, hh * QC:(hh + 1) * QC],
                            kt_sb[p][pb:pb + 64, mt * P:(mt + 1) * P],
                            qt_sb[p][pb:pb + 64, q0:q0 + QC],
                            start=True, stop=True,
                        )
                    es = es_pool.tile([P, 2 * QC], bf16, tag="es", name=f"es{qc}_{p}_{mt}")
                    nc.scalar.activation(
                        es[:], s_t[:],
                        mybir.ActivationFunctionType.Exp, scale=SCALE,
                    )
                    fill(qc, p, mt)
                    for hh in range(2):
                        h = 2 * p + hh
                        nc.tensor.matmul(
                            o_ts[hh][:],
                            v_sb[:, mt, h, :],
                            es[:, hh * QC:(hh + 1) * QC],
                            start=(mt == 0), stop=(mt == MT - 1),
                        )
                normalize(qc, p, o_ts)

    # ---- tail: final projection for qc3 ----
    # ao p0 is ready well before ao p1 (its normalize ends the kernel), so:
    # j0 accumulation steps first (4 open banks), junk bridge keeps the PE
    # pstate warm while normalize(qc3,p1) completes, then the j1 steps.
    with (
        tc.tile_pool(name="tail_ps", bufs=7, space="PSUM") as tps,
        tc.tile_pool(name="tailj_ps", bufs=1, space="PSUM") as tjp,
    ):
        fts = {}
        ostgs = {}

        def fin3_mm(i, j):
            nt = 12 + i // 2
            ec = i % 2
            if j == 0:
                fts[i] = tps.tile([P, QC], f32, tag="tf", name=f"tf{i}")
            nc.tensor.matmul(
                fts[i][:],
                ao_sb[j][:, nt * P:(nt + 1) * P],
                wo_sb[:, j, ec * QC:(ec + 1) * QC],
                start=(j == 0), stop=(j == 1),
            )
            if j == 0 and ec == 0:
                ostgs[nt] = outst_pool.tile([P, C], bf16, tag="ostg", name=f"og{nt}")
            if j == 1:
                nc.vector.tensor_copy(ostgs[nt][:, ec * QC:(ec + 1) * QC], fts[i][:])
                deng = nc.sync if ec == 1 else nc.gpsimd
                deng.dma_start(out[nt * P:(nt + 1) * P, ec * QC:(ec + 1) * QC],
                               ostgs[nt][:, ec * QC:(ec + 1) * QC])

        for i in range(4):
            fin3_mm(i, 0)
        emit_junk(tjp, opt.get("tail_junk", 13), tag="warm1")
        for i in range(4):
            fin3_mm(i, 1)
        for i in range(4, 8):
            fin3_mm(i, 0)
        for i in (6, 7, 4, 5):   # last row group first so its copy+DMA drain early
            fin3_mm(i, 1)


def _build(reps=1, opt=None):
    key = (reps, tuple(sorted((opt or {}).items())))
    if key in _CACHE:
        return _CACHE[key]
    nc = bacc.Bacc("TRN2", target_bir_lowering=False, debug=False)
    xd = nc.dram_tensor("xd", [P, KT_TILES, N], bf16, kind="ExternalInput")
    ctxd = nc.dram_tensor("ctxd", [P, KT_TILES, M], bf16, kind="ExternalInput")
    wq = nc.dram_tensor("wq", [P, KT_TILES, IC], bf16, kind="ExternalInput")
    wk = nc.dram_tensor("wk", [P, KT_TILES, IC], bf16, kind="ExternalInput")
    wv = nc.dram_tensor("wv", [P, KT_TILES, IC], bf16, kind="ExternalInput")
    wo = nc.dram_tensor("wo", [P, 2, C], bf16, kind="ExternalInput")
    out = nc.dram_tensor("out", [N, C], bf16, kind="ExternalOutput")
    with tile.TileContext(nc) as tc:
        for _ in range(reps):
            with ExitStack() as ctx:
                _body(nc, tc, ctx, xd, ctxd, wq, wk, wv, wo, out, opt=opt)
    nc.compile()
    _CACHE[key] = nc
    return nc


def _to_tiled(a, inner):
    """[K*128, inner] f32 -> [128, K, inner] bf16 (partition-major tiling)."""
    k = a.shape[0] // P
    return np.ascontiguousarray(
        a.reshape(k, P, inner).transpose(1, 0, 2).astype(ml_dtypes.bfloat16)
    )


def _shard_inputs(x, context, Wq, Wk, Wv, Wo):
    in_maps = []
    for c in range(NCORES):
        b, g = divmod(c, NCORES // B)
        cols = slice(g * IC, (g + 1) * IC)
        in_maps.append({
            "xd": _to_tiled(np.ascontiguousarray(x[b].T), N),
            "ctxd": _to_tiled(np.ascontiguousarray(context[b].T), M),
            "wq": _to_tiled(np.ascontiguousarray(Wq[:, cols]), IC),
            "wk": _to_tiled(np.ascontiguousarray(Wk[:, cols]), IC),
            "wv": _to_tiled(np.ascontiguousarray(Wv[:, cols]), IC),
            "wo": _to_tiled(np.ascontiguousarray(Wo[cols, :]), C),
        })
    return in_maps


def kernel(x, context, Wq, Wk, Wv, Wo, reps=1):
    x = np.asarray(x, dtype=np.float32)
    context = np.asarray(context, dtype=np.float32)
    Wq, Wk, Wv, Wo = (np.asarray(w, dtype=np.float32) for w in (Wq, Wk, Wv, Wo))
    nc = _build(reps)
    in_maps = _shard_inputs(x, context, Wq, Wk, Wv, Wo)
    res = run_bass_kernel_spmd(nc, in_maps, core_ids=list(range(NCORES)))
    gpb = NCORES // B
    out = np.zeros((B, N, C), dtype=np.float32)
    for c in range(NCORES):
        out[c // gpb] += np.asarray(res.results[c]["out"], dtype=np.float32)
    return out


# revision 35
# speedup vs baseline: 1.0572x; 1.0135x over previous
"""Trainium2 Bass kernel for CrossAttention (B=2, N=M=2048, 16 heads x 64).

Sharding: batch x head-group parallel over 8 cores. Core c handles batch
c//4 and heads [4*(c%4), 4*(c%4)+4). Projection weights are column-split
(Wq/Wk/Wv) / row-split (Wo) per core; each core produces a partial
[2048, 1024] output (bf16) which the host sums per batch (4 partials).

V2 design (cost-model driven):
  - All DRAM inputs and SBUF matmul operands are bf16 (same 1 cycle/row
    PE speed as f32r, half the DMA bytes, ~5e-3 rel err total).
  - ACT engine runs ONLY the softmax Exp (131072 rows = the 133us floor);
    all DMAs/copies live on SP/DVE/Pool queues.
  - PE is the bottleneck (393216 matmul rows = 163.8us floor).  Emission
    keeps PE saturated: KT -> QT(qc0) -> per q-chunk/head-pair S/exp/O
    streams with "fill" matmuls (QT for later q-chunks, fused output
    projection of earlier q-chunks, V during the first chunk) interleaved
    one per m-tile to absorb the exp-vs-PE rate gap.
  - Normalization: ones column in V_aug yields denominators in PSUM row
    64; DVE reciprocal, SP DMA to partition 0, gpsimd partition_broadcast,
    DVE multiply (bf16 out).  PSUM: 2x s_t (2 banks each) + 3x o_t +
    1 fill bank = 8 banks.
"""

import numpy as np
import ml_dtypes
from contextlib import ExitStack

import concourse.tile as tile
from concourse import bacc, mybir
from concourse.bass_utils import run_bass_kernel_spmd

B, N, M, C = 2, 2048, 2048, 1024
HEADS, D = 16, 64
HPC = 4            # heads per core
IC = HPC * D       # 256 inner dims per core
SCALE = D ** -0.5
NCORES = 8
KT_TILES = C // 128   # 8 contraction tiles for projections
P = 128
MT = M // P           # 16 m tiles
QC = 512
NQC = N // QC         # 4 q chunks
f32 = mybir.dt.float32
bf16 = mybir.dt.bfloat16

_CACHE = {}


def _body(nc, tc, ctx, xd, ctxd, wq, wk, wv, wo, out, opt=None):
    opt = opt or {}
    ES_BUFS = opt.get("es_bufs", 4)
    O_BUFS = opt.get("o_bufs", 2)
    F_BUFS = opt.get("f_bufs", 2)
    BCAST64 = opt.get("bcast64", False)
    GP_AOPACK = opt.get("gp_aopack", True)
    ONES0 = opt.get("ones0", True)

    const = ctx.enter_context(tc.tile_pool(name="const", bufs=1))
    wq_sb = const.tile([P, KT_TILES, IC], bf16, tag="wq")
    wk_sb = const.tile([P, KT_TILES, IC], bf16, tag="wk")
    wv_sb = const.tile([P, KT_TILES, IC], bf16, tag="wv")
    wo_sb = const.tile([P, 2, C], bf16, tag="wo")
    ctx_sb = const.tile([P, KT_TILES, M], bf16, tag="ctx")
    x_sb = const.tile([P, KT_TILES, N], bf16, tag="x")
    kt_sb = [const.tile([P, M], bf16, tag=f"kt{j}", name=f"kt{j}") for j in range(2)]
    qt_sb = [const.tile([P, N], bf16, tag=f"qt{j}", name=f"qt{j}") for j in range(2)]
    VW = 2 * D  # ones | zeros pad | V values
    v_sb = const.tile([P, MT, HPC, VW], bf16, tag="v")
    ao_sb = [const.tile([P, N], bf16, tag=f"ao{j}", name=f"ao{j}") for j in range(2)]
    ones_sb = const.tile([P, 1], f32, tag="ones")

    es_pool = ctx.enter_context(tc.tile_pool(name="es", bufs=ES_BUFS))
    r_pool = ctx.enter_context(tc.tile_pool(name="rp", bufs=2))
    rb_pool = ctx.enter_context(tc.tile_pool(name="rbp", bufs=2))
    oc_pool = ctx.enter_context(tc.tile_pool(name="ocp", bufs=4))
    aot_pool = ctx.enter_context(tc.tile_pool(name="aot", bufs=2))
    outst_pool = ctx.enter_context(tc.tile_pool(name="outst", bufs=4))

    junk_sb = const.tile([P, QC], bf16, tag="junk")

    # junk tile first on DVE so PE warmup matmuls can start ~1us in
    nc.vector.memset(junk_sb[:], 0.0)
    # ones column of V_aug (f32 memset + broadcast-copy cast to bf16)
    nc.vector.memset(ones_sb[:], 1.0)
    nc.vector.memset(v_sb[:, :, :, 1:D], 0.0)
    nc.vector.tensor_copy(
        v_sb[:, :, :, 0:1],
        ones_sb[:, 0:1].to_broadcast((P, MT, HPC, 1)),
    )

    def emit_junk(pool, n, tag="warm"):
        # PE keep-warm matmuls (nothing reads the result): bridge idle
        # windows so the pstate ramp does not reset.
        jp = pool.tile([P, QC], f32, tag=tag, name=f"junk_{tag}")
        for _ in range(n):
            nc.tensor.matmul(
                jp[:], junk_sb[0:P, 0:P], junk_sb[:, 0:QC],
                start=True, stop=True,
            )

    # ---- input DMAs ----
    # ACT: weights + x qc0 (all done before the first exp); SP: ctx + x rest
    # first wk k-tile + ctx k0 quarters split so the first KT matmul starts early
    for k in range(KT_TILES):
        nc.scalar.dma_start(wq_sb[:, k, :], wq[:, k, :])
        nc.scalar.dma_start(x_sb[:, k, 0:QC], xd[:, k, 0:QC])
        nc.scalar.dma_start(wk_sb[:, k, :], wk[:, k, :])
    for k in range(KT_TILES):
        eng = nc.sync if k % 2 == 0 else nc.gpsimd
        eng.dma_start(ctx_sb[:, k, :], ctxd[:, k, :])
    nc.scalar.dma_start(wv_sb[:], wv[:])
    nc.scalar.dma_start(wo_sb[:], wo[:])
    for k in range(KT_TILES):
        nc.sync.dma_start(x_sb[:, k, QC:N], xd[:, k, QC:N])

    def emit_V(m, pool):
        vt = pool.tile([P, QC], f32, tag=pool_tag[id(pool)], name=f"vt{m}")
        for k in range(KT_TILES):
            nc.tensor.matmul(
                vt[:, 0:IC],
                ctx_sb[:, k, m * P:(m + 1) * P],
                wv_sb[:, k, :],
                start=(k == 0), stop=(k == KT_TILES - 1),
            )
        nc.vector.tensor_copy(
            v_sb[:, m, :, D:VW],
            vt[:, 0:IC].rearrange("p (h d) -> p h d", d=D),
        )

    pool_tag = {}

    # ---- PE warmup, then projections: KT (8 psum banks, k-outer), QT qc0 ----
    with tc.tile_pool(name="warm_ps", bufs=1, space="PSUM") as wps:
        emit_junk(wps, 6, tag="warm0")

    with tc.tile_pool(name="pp8", bufs=8, space="PSUM") as pps:
        pool_tag[id(pps)] = "pp"
        # QT qc0 (both j) and KT j0 interleaved per k-tile, tracking the
        # per-k weight/x/ctx DMA arrival order.
        qp = [pps.tile([P, QC], f32, tag="pp", name=f"qp{j}") for j in range(2)]
        kp0 = [pps.tile([P, QC], f32, tag="pp", name=f"kp0_{qc}") for qc in range(4)]
        for k in range(KT_TILES):
            for j in range(2):
                nc.tensor.matmul(
                    qp[j][:],
                    wq_sb[:, k, j * P:(j + 1) * P],
                    x_sb[:, k, 0:QC],
                    start=(k == 0), stop=(k == KT_TILES - 1),
                )
            for qc in range(4):
                nc.tensor.matmul(
                    kp0[qc][:],
                    wk_sb[:, k, 0:P],
                    ctx_sb[:, k, qc * QC:(qc + 1) * QC],
                    start=(k == 0), stop=(k == KT_TILES - 1),
                )
        nc.vector.tensor_copy(qt_sb[0][:, 0:QC], qp[0][:])
        nc.vector.tensor_copy(qt_sb[1][:, 0:QC], qp[1][:])
        for qc in range(4):
            nc.vector.tensor_copy(kt_sb[0][:, qc * QC:(qc + 1) * QC], kp0[qc][:])
        # KT j1 qc-major (ctx fully resident by now); last two chunks reuse
        # the QT banks, drained by the qt copies above.
        for qc in range(4):
            kp1 = pps.tile([P, QC], f32, tag="pp", name=f"kp1_{qc}")
            for k in range(KT_TILES):
                nc.tensor.matmul(
                    kp1[:],
                    wk_sb[:, k, P:2 * P],
                    ctx_sb[:, k, qc * QC:(qc + 1) * QC],
                    start=(k == 0), stop=(k == KT_TILES - 1),
                )
            nc.vector.tensor_copy(kt_sb[1][:, qc * QC:(qc + 1) * QC], kp1[:])
        # V[0]: covers the kt j1 copies + attention pool transition
        emit_V(0, pps)

    # ---- attention with interleaved fill work ----
    with ExitStack() as attn_ctx:
        sps = attn_ctx.enter_context(tc.tile_pool(name="s_ps", bufs=2, space="PSUM"))
        ops = attn_ctx.enter_context(tc.tile_pool(name="o_ps", bufs=O_BUFS, space="PSUM"))
        fps = attn_ctx.enter_context(tc.tile_pool(name="f_ps", bufs=F_BUFS, space="PSUM"))
        pool_tag[id(fps)] = "f"

        # QT fill state: one matmul per fill slot, j-serial per q-chunk
        qt_state = {}

        def emit_QT(qcn, slot):
            j, k = slot // KT_TILES, slot % KT_TILES
            if k == 0:
                qt_state[(qcn, j)] = fps.tile([P, QC], f32, tag="f", name=f"qf{qcn}_{j}")
            qtile = qt_state[(qcn, j)]
            nc.tensor.matmul(
                qtile[:],
                wq_sb[:, k, j * P:(j + 1) * P],
                x_sb[:, k, qcn * QC:(qcn + 1) * QC],
                start=(k == 0), stop=(k == KT_TILES - 1),
            )
            if k == KT_TILES - 1:
                nc.vector.tensor_copy(qt_sb[j][:, qcn * QC:(qcn + 1) * QC], qtile[:])
                del qt_state[(qcn, j)]

        # final projection fill: slot -> (nt, ec, j)
        fin_state = {}

        def emit_FIN(qcn, slot, pool):
            nt = qcn * 4 + slot // 4
            ec = (slot // 2) % 2
            j = slot % 2
            if j == 0:
                fin_state["ft"] = pool.tile(
                    [P, QC], f32, tag=pool_tag[id(pool)], name=f"ft{nt}_{ec}")
            ft = fin_state["ft"]
            nc.tensor.matmul(
                ft[:],
                ao_sb[j][:, nt * P:(nt + 1) * P],
                wo_sb[:, j, ec * QC:(ec + 1) * QC],
                start=(j == 0), stop=(j == 1),
            )
            if j == 0 and ec == 0:
                fin_state["ostg"] = outst_pool.tile([P, C], bf16, tag="ostg",
                                                    name=f"og{nt}")
            if j == 1:
                nc.vector.tensor_copy(fin_state["ostg"][:, ec * QC:(ec + 1) * QC], ft[:])
                if ec == 1:
                    deng = nc.sync if nt % 2 == 0 else nc.gpsimd
                    deng.dma_start(out[nt * P:(nt + 1) * P, :], fin_state["ostg"][:])

        # fill plan per (qc, p) segment
        def fill(qc, p, mt):
            if qc == 0 and p == 0:
                if mt + 1 < MT:
                    emit_V(mt + 1, fps)
            elif qc == 0 and p == 1:
                emit_QT(1, mt)
            elif qc == 1 and p == 0:
                emit_QT(2, mt)
            elif qc == 1 and p == 1:
                emit_FIN(0, mt, fps)
            elif qc == 2 and p == 0:
                emit_QT(3, mt)
            elif qc == 2 and p == 1:
                emit_FIN(1, mt, fps)
            elif qc == 3 and p == 0:
                emit_FIN(2, mt, fps)
            # (3,1): no fill available

        def normalize(qc, p, o_ts):
            # Chain (hh1 first; it gates the fused output projection):
            #   DVE: recip1 (straight from PSUM), oc1 copy, mul1, recip0, mul0
            #   Pool: bcast1, oc0 copy, bcast0
            # o banks freed by {recip, oc copy}; ao written by muls/pack-DMA.
            q0 = qc * QC
            rs, rbs, ocs = {}, {}, {}
            for hh in (1, 0):
                rs[hh] = r_pool.tile([P, QC], f32, tag="r", name=f"r{qc}_{p}_{hh}")
                rbs[hh] = rb_pool.tile([P, QC], f32, tag="rb", name=f"rb{qc}_{p}_{hh}")
                ocs[hh] = oc_pool.tile([P, QC], f32, tag="oc", name=f"oc{qc}_{p}_{hh}")

            def bcast(hh):
                if BCAST64:
                    nc.gpsimd.partition_broadcast(rbs[hh][0:D, :], rs[hh][64:65, :])
                else:
                    nc.sync.dma_start(rs[hh][0:1, :], rs[hh][64:65, :])
                    nc.gpsimd.partition_broadcast(rbs[hh][0:D, :], rs[hh][0:1, :])

            if qc == NQC - 1 and p == 1:
                nc.scalar.copy(ocs[1][:], o_ts[1][:])
                nc.scalar.copy(ocs[0][:], o_ts[0][:])
            else:
                nc.vector.tensor_copy(ocs[1][:], o_ts[1][:])
                nc.vector.tensor_copy(ocs[0][:], o_ts[0][:])
            nc.vector.reciprocal(rs[1][0:1, :], ocs[1][0:1, :])
            nc.vector.reciprocal(rs[0][0:1, :], ocs[0][0:1, :])
            nc.gpsimd.partition_broadcast(rbs[1][0:D, :], rs[1][0:1, :])
            nc.gpsimd.partition_broadcast(rbs[0][0:D, :], rs[0][0:1, :])
            nc.gpsimd.tensor_mul(
                ao_sb[p][64:P, q0:q0 + QC], ocs[1][D:2 * D, :], rbs[1][0:D, :]
            )
            nc.gpsimd.tensor_mul(
                ao_sb[p][0:D, q0:q0 + QC], ocs[0][D:2 * D, :], rbs[0][0:D, :]
            )

        for qc in range(NQC):
            q0 = qc * QC
            for p in range(2):
                o_ts = [ops.tile([P, QC], f32, tag="o", name=f"o{qc}_{p}_{i}")
                        for i in range(2)]
                if qc == 0 and p == 0:
                    pass  # V[0], V[1] were emitted in the projection scope
                for mt in range(MT):
                    s_t = sps.tile([P, 2 * QC], f32, tag="s", name=f"s{qc}_{p}_{mt}")
                    for hh in range(2):
                        pb = hh * 64
                        nc.tensor.matmul(
                            s_t[:, hh * QC:(hh + 1) * QC],
                            kt_sb[p][pb:pb + 64, mt * P:(mt + 1) * P],
                            qt_sb[p][pb:pb + 64, q0:q0 + QC],
                            start=True, stop=True,
                        )
                    es = es_pool.tile([P, 2 * QC], bf16, tag="es", name=f"es{qc}_{p}_{mt}")
                    nc.scalar.activation(
                        es[:], s_t[:],
                        mybir.ActivationFunctionType.Exp, scale=SCALE,
                    )
                    fill(qc, p, mt)
                    for hh in range(2):
                        h = 2 * p + hh
                        nc.tensor.matmul(
                            o_ts[hh][:],
                            v_sb[:, mt, h, :],
                            es[:, hh * QC:(hh + 1) * QC],
                            start=(mt == 0), stop=(mt == MT - 1),
                        )
                normalize(qc, p, o_ts)

    # ---- tail: final projection for qc3 ----
    # ao p0 is ready well before ao p1 (its normalize ends the kernel), so:
    # j0 accumulation steps first (4 open banks), junk bridge keeps the PE
    # pstate warm while normalize(qc3,p1) completes, then the j1 steps.
    with (
        tc.tile_pool(name="tail_ps", bufs=7, space="PSUM") as tps,
        tc.tile_pool(name="tailj_ps", bufs=1, space="PSUM") as tjp,
    ):
        fts = {}
        ostgs = {}

        def fin3_mm(i, j):
            nt = 12 + i // 2
            ec = i % 2
            if j == 0:
                fts[i] = tps.tile([P, QC], f32, tag="tf", name=f"tf{i}")
            nc.tensor.matmul(
                fts[i][:],
                ao_sb[j][:, nt * P:(nt + 1) * P],
                wo_sb[:, j, ec * QC:(ec + 1) * QC],
                start=(j == 0), stop=(j == 1),
            )
            if j == 0 and ec == 0:
                ostgs[nt] = outst_pool.tile([P, C], bf16, tag="ostg", name=f"og{nt}")
            if j == 1:
                ceng = nc.scalar if ec == 0 else nc.vector
                if ec == 0:
                    nc.scalar.copy(ostgs[nt][:, ec * QC:(ec + 1) * QC], fts[i][:])
                else:
                    nc.vector.tensor_copy(ostgs[nt][:, ec * QC:(ec + 1) * QC], fts[i][:])
                deng = nc.sync if ec == 1 else nc.gpsimd
                deng.dma_start(out[nt * P:(nt + 1) * P, ec * QC:(ec + 1) * QC],
                               ostgs[nt][:, ec * QC:(ec + 1) * QC])

        for i in range(4):
            fin3_mm(i, 0)
        emit_junk(tjp, opt.get("tail_junk", 13), tag="warm1")
        for i in range(4):
            fin3_mm(i, 1)
        for i in range(4, 8):
            fin3_mm(i, 0)
        for i in (6, 7, 4, 5):   # last row group first so its copy+DMA drain early
            fin3_mm(i, 1)


def _build(reps=1, opt=None):
    key = (reps, tuple(sorted((opt or {}).items())))
    if key in _CACHE:
        return _CACHE[key]
    nc = bacc.Bacc("TRN2", target_bir_lowering=False, debug=False)
    xd = nc.dram_tensor("xd", [P, KT_TILES, N], bf16, kind="ExternalInput")
    ctxd = nc.dram_tensor("ctxd", [P, KT_TILES, M], bf16, kind="ExternalInput")
    wq = nc.dram_tensor("wq", [P, KT_TILES, IC], bf16, kind="ExternalInput")
    wk = nc.dram_tensor("wk", [P, KT_TILES, IC], bf16, kind="ExternalInput")
    wv = nc.dram_tensor("wv", [P, KT_TILES, IC], bf16, kind="ExternalInput")
    wo = nc.dram_tensor("wo", [P, 2, C], bf16, kind="ExternalInput")
    out = nc.dram_tensor("out", [N, C], bf16, kind="ExternalOutput")
    with tile.TileContext(nc) as tc:
        for _ in range(reps):
            with ExitStack() as ctx:
                _body(nc, tc, ctx, xd, ctxd, wq, wk, wv, wo, out, opt=opt)
    nc.compile()
    _CACHE[key] = nc
    return nc


def _to_tiled(a, inner):
    """[K*128, inner] f32 -> [128, K, inner] bf16 (partition-major tiling)."""
    k = a.shape[0] // P
    return np.ascontiguousarray(
        a.reshape(k, P, inner).transpose(1, 0, 2).astype(ml_dtypes.bfloat16)
    )


def _shard_inputs(x, context, Wq, Wk, Wv, Wo):
    in_maps = []
    for c in range(NCORES):
        b, g = divmod(c, NCORES // B)
        cols = slice(g * IC, (g + 1) * IC)
        in_maps.append({
            "xd": _to_tiled(np.ascontiguousarray(x[b].T), N),
            "ctxd": _to_tiled(np.ascontiguousarray(context[b].T), M),
            "wq": _to_tiled(np.ascontiguousarray(Wq[:, cols]), IC),
            "wk": _to_tiled(np.ascontiguousarray(Wk[:, cols]), IC),
            "wv": _to_tiled(np.ascontiguousarray(Wv[:, cols]), IC),
            "wo": _to_tiled(np.ascontiguousarray(Wo[cols, :]), C),
        })
    return in_maps


def kernel(x, context, Wq, Wk, Wv, Wo, reps=1):
    x = np.asarray(x, dtype=np.float32)
    context = np.asarray(context, dtype=np.float32)
    Wq, Wk, Wv, Wo = (np.asarray(w, dtype=np.float32) for w in (Wq, Wk, Wv, Wo))
    nc = _build(reps)
    in_maps = _shard_inputs(x, context, Wq, Wk, Wv, Wo)
    res = run_bass_kernel_spmd(nc, in_maps, core_ids=list(range(NCORES)))
    gpb = NCORES // B
    out = np.zeros((B, N, C), dtype=np.float32)
    for c in range(NCORES):
        out[c // gpb] += np.asarray(res.results[c]["out"], dtype=np.float32)
    return out
